# revision 19
# baseline (speedup 1.0000x reference)
"""Expert-parallel + token-parallel Trainium2 kernel for ComplexityDecoderLayerV2.

Distribution (8 cores, E=4 experts):
  - Tokens are pre-sorted by expert on the host (sort_idx); core c owns the
    256 sorted tokens S_c = sort_idx[256c:256c+256] (expert e = c//2) and all
    per-token compute for them: input rmsnorm, routed q projection, its slice
    of K/V, all 16 attention heads for its 256 query tokens, routed o
    projection, residuals, mu projection, post norm, and the routed MLP.
  - K/V are computed per-core for the core's tokens, then AllGather'd
    (replacing the reference's gather/scatter with a collective, per the
    sharding hint). Causality over the shuffled token order is enforced with a
    host-built multiplicative {0,1} mask on the scores.
  - Activations are kept feature-major ([feature, token]) on-chip so every
    matmul chains without transposes; host transposes/un-permutes at the
    boundary (pure data movement).
Compute dtype: float32r matmuls (fp32 storage, ~1e-4 matmul rel err) with a
bf16 softmax/V path; all accumulation in fp32 PSUM.
"""

import sys

if "/opt/trn_rl_repo" not in sys.path:
    sys.path.insert(0, "/opt/trn_rl_repo")

import numpy as np
import ml_dtypes

import concourse.bass as bass
import concourse.mybir as mybir
import concourse.tile as tile
from concourse import bacc
from concourse.bass_utils import run_bass_kernel_spmd
from concourse.tile import add_dep_helper

f32 = mybir.dt.float32
f32r = mybir.dt.float32r
bf16 = mybir.dt.bfloat16
AF = mybir.ActivationFunctionType
ALU = mybir.AluOpType

NC = 8            # cores
N = 2048          # tokens
D = 2048          # hidden
NH = 16           # q heads
NKV = 4           # kv heads
HD = 128          # head dim
E = 4             # experts
EI = 2048         # per-expert intermediate
T = N // NC       # tokens per core = 256
DT = D // 128     # 16 feature tiles
EPS = 1e-6
THETA = 10000.0

_cached = {}


def _build():
    nc = bacc.Bacc("TRN2", target_bir_lowering=False, debug=False, num_devices=NC)

    # ---- per-core external inputs ----
    xT_d = nc.dram_tensor("xT", [D, T], f32, kind="ExternalInput")
    cos_d = nc.dram_tensor("cosT", [64, T], f32, kind="ExternalInput")
    sin_d = nc.dram_tensor("sinT", [64, T], f32, kind="ExternalInput")
    mask_d = nc.dram_tensor("maskb", [N, T], bf16, kind="ExternalInput")
    lnin_d = nc.dram_tensor("lnw_in", [128, DT], f32, kind="ExternalInput")
    lnpost_d = nc.dram_tensor("lnw_post", [128, DT], f32, kind="ExternalInput")
    qnw_d = nc.dram_tensor("qnw", [128, 1], f32, kind="ExternalInput")
    knw_d = nc.dram_tensor("knw", [128, 1], f32, kind="ExternalInput")
    mu_d = nc.dram_tensor("muvec", [128, DT], f32, kind="ExternalInput")
    wq_d = nc.dram_tensor("wq", [D, NH * HD], f32r, kind="ExternalInput")
    wo_d = nc.dram_tensor("wo", [NH * HD, D], f32r, kind="ExternalInput")
    wgu_d = nc.dram_tensor("wgu", [D, 2 * EI], f32r, kind="ExternalInput")
    wdw_d = nc.dram_tensor("wdw", [EI, D], f32r, kind="ExternalInput")
    wmuT_d = nc.dram_tensor("wmuT", [D, D], f32r, kind="ExternalInput")
    wkT_d = nc.dram_tensor("wkT", [D, NKV * HD], f32r, kind="ExternalInput")
    wvT_d = nc.dram_tensor("wvT", [D, NKV * HD], f32r, kind="ExternalInput")
    hout_d = nc.dram_tensor("h_out", [D, T], f32, kind="ExternalOutput")
    muout_d = nc.dram_tensor("mu_out", [D, T], f32, kind="ExternalOutput")
    DEBUG = bool(__import__("os").environ.get("KERNEL_DEBUG"))
    if DEBUG:
        dbg_xn = nc.dram_tensor("dbg_xn", [D, T], f32, kind="ExternalOutput")
        dbg_q = nc.dram_tensor("dbg_q", [D, T], f32, kind="ExternalOutput")
        dbg_at = nc.dram_tensor("dbg_at", [D, T], f32, kind="ExternalOutput")
        dbg_h1 = nc.dram_tensor("dbg_h1", [D, T], f32, kind="ExternalOutput")
        dbg_k = nc.dram_tensor("dbg_k", [NKV * HD, T], f32, kind="ExternalOutput")
        dbg_v = nc.dram_tensor("dbg_v", [T, NKV * HD], f32, kind="ExternalOutput")

    from contextlib import ExitStack

    with tile.TileContext(nc) as tc, \
            nc.allow_low_precision(reason="f32r/bf16 compute by design"), \
            ExitStack() as stk:
        rg = [list(range(NC))]

        perm = stk.enter_context(tc.tile_pool(name="perm", bufs=1))
        wpool = stk.enter_context(tc.tile_pool(name="wpool", bufs=6))
        sqp = stk.enter_context(tc.tile_pool(name="sqp", bufs=2))
        rp = stk.enter_context(tc.tile_pool(name="rp", bufs=8))
        vec = stk.enter_context(tc.tile_pool(name="vec", bufs=2))
        psum = stk.enter_context(tc.tile_pool(name="psum", bufs=2, space="PSUM"))
        dram = stk.enter_context(tc.tile_pool(name="dram", bufs=1, space="DRAM"))

        if True:
            # ---- constants ----
            epsA = perm.tile([1, 1], f32)
            nc.vector.memset(epsA[:], EPS)
            epsB = perm.tile([1, 1], f32)
            nc.vector.memset(epsB[:], 128.0 * EPS)
            ones_f = perm.tile([128, 1], f32)
            nc.vector.memset(ones_f[:], 1.0)
            ones_r = perm.tile([128, 1], f32r)
            nc.vector.tensor_copy(ones_r[:], ones_f[:])
            ones1_f = perm.tile([1, 128], f32)
            nc.vector.memset(ones1_f[:], 1.0)
            ones1_r = perm.tile([1, 128], f32r)
            nc.vector.tensor_copy(ones1_r[:], ones1_f[:])
            ones_b = perm.tile([128, 1], bf16)
            nc.vector.memset(ones_b[:], 1.0)
            lnin = perm.tile([128, DT], f32)
            nc.sync.dma_start(lnin[:], lnin_d[:])
            lnpost = perm.tile([128, DT], f32)
            nc.sync.dma_start(lnpost[:], lnpost_d[:])
            qnw = perm.tile([128, 1], f32)
            nc.sync.dma_start(qnw[:], qnw_d[:])
            knw = perm.tile([128, 1], f32)
            nc.sync.dma_start(knw[:], knw_d[:])
            muv = perm.tile([128, DT], f32)
            nc.sync.dma_start(muv[:], mu_d[:])
            muclip = perm.tile([128, DT], f32)
            nc.vector.tensor_scalar(muclip[:], muv[:], 0.0, 2.0, ALU.max, ALU.min)
            cosT = perm.tile([64, T], f32)
            nc.sync.dma_start(cosT[:], cos_d[:])
            sinT = perm.tile([64, T], f32)
            nc.sync.dma_start(sinT[:], sin_d[:])
            cs1 = perm.tile([128, T], f32)
            nc.vector.tensor_copy(cs1[0:64, :], cosT[:])
            nc.vector.tensor_copy(cs1[64:128, :], sinT[:])
            cs2 = perm.tile([128, T], f32)
            nc.vector.tensor_copy(cs2[0:64, :], sinT[:])
            nc.vector.tensor_copy(cs2[64:128, :], cosT[:])

            # ---- persistent activations ----
            xT = perm.tile([128, DT * T], f32)
            for i in range(DT):
                nc.sync.dma_start(xT[:, T * i:T * (i + 1)], xT_d[128 * i:128 * (i + 1), :])
            qT = perm.tile([128, NH * T], f32r)
            attnT = perm.tile([128, NH * T], f32r)
            h1T = perm.tile([128, DT * T], f32r)

            # collective bounce buffers
            cc_in_k = dram.tile([NKV * HD, T], f32r)
            cc_in_v = dram.tile([T, NKV * HD], bf16)
            cc_out_k = dram.tile([NC * NKV * HD, T], f32r, addr_space="Shared")
            cc_out_v = dram.tile([N, NKV * HD], bf16, addr_space="Shared")

            def rnorm_vec(ss_ap, bias_t, scale):
                """1/sqrt(scale*ss + bias) as a [1, T] f32r vector + [128, T] psum bcast."""
                s_t = vec.tile([1, T], f32, tag="s", name=f"s_{nc.next_id()}")
                nc.scalar.activation(s_t[:], ss_ap, AF.Sqrt, bias=bias_t[:], scale=scale)
                inv_t = vec.tile([1, T], f32r, tag="inv", name=f"inv_{nc.next_id()}")
                nc.vector.reciprocal(inv_t[:], s_t[:])
                return inv_t

            # ================= phase A: input rmsnorm =================
            gA = psum.tile([128, 2048], f32, tag="g", name="gA")
            ssA = gA[0:1, 0:T]
            xsqB = sqp.tile([128, DT * T], f32r, tag="sqB", name="xsqB", bufs=1)
            nc.vector.tensor_tensor(xsqB[:], xT[:], xT[:], ALU.mult)
            for i in range(DT):
                nc.tensor.matmul(ssA, ones_r[:], xsqB[:, T * i:T * (i + 1)], start=(i == 0), stop=(i == DT - 1),
                                 skip_group_check=True)
            invA = rnorm_vec(ssA, epsA, 1.0 / D)
            invbA = gA[:, T:2 * T]
            nc.tensor.matmul(invbA, ones1_r[:], invA[:], start=True, stop=True,
                             skip_group_check=True)
            with tc.tile_pool(name="pre", bufs=1) as pre:
                xnT = pre.tile([128, DT * T], f32r)
                for i in range(DT):
                    nc.vector.scalar_tensor_tensor(
                        xnT[:, T * i:T * (i + 1)], xT[:, T * i:T * (i + 1)],
                        lnin[:, i:i + 1], invbA, ALU.mult, ALU.mult)

                if DEBUG:
                    for i in range(DT):
                        nc.sync.dma_start(dbg_xn[128 * i:128 * (i + 1), :], xnT[:, T * i:T * (i + 1)].bitcast(f32))

                # ================= phase A: K/V projections (first, to launch AG early) ====
                kraw = pre.tile([128, NKV * T], f32)
                gK = psum.tile([128, 2048], f32, tag="g", name="gK")
                for i in range(DT):
                    wk = wpool.tile([128, 2048], f32r, tag="w", name=f"wk{i}")
                    nc.sync.dma_start(wk[:, 0:NKV * HD], wkT_d[128 * i:128 * (i + 1), :])
                    for j in range(NKV):
                        mm = nc.tensor.matmul(gK[:, T * j:T * (j + 1)], wk[:, 128 * j:128 * (j + 1)],
                                              xnT[:, T * i:T * (i + 1)],
                                              start=(i == 0 and j % 2 == 0), stop=(i == DT - 1),
                                              skip_group_check=True)
                        if i == 0:
                            if j % 2 == 0:
                                bank_start = mm
                            else:
                                add_dep_helper(mm.ins, bank_start.ins, sync=False,
                                               reason="psum bank pair order")
                for j in range(NKV):
                    nc.vector.tensor_copy(kraw[:, T * j:T * (j + 1)], gK[:, T * j:T * (j + 1)])

                # k rmsnorm + rope (all 4 heads batched), then to bounce
                kT_c = pre.tile([128, NKV * T], f32r)
                ksq = sqp.tile([128, NKV * T], f32r, tag="sq4", name="ksq", bufs=2)
                nc.vector.tensor_tensor(ksq[:], kraw[:], kraw[:], ALU.mult)
                gn = psum.tile([128, 2048], f32, tag="g", name="gkn")
                ssk = gn[0:1, 0:NKV * T]
                nc.tensor.matmul(gn[0:1, 0:512], ones_r[:], ksq[:, 0:512], start=True, stop=True,
                                 skip_group_check=True)
                nc.tensor.matmul(gn[0:1, 512:1024], ones_r[:], ksq[:, 512:1024], start=True, stop=True,
                                 skip_group_check=True)
                sk = vec.tile([1, NKV * T], f32, tag="s4", name="sk")
                nc.scalar.activation(sk[:], ssk, AF.Sqrt, bias=epsA[:], scale=1.0 / HD)
                invk = vec.tile([1, NKV * T], f32r, tag="inv4", name="invk")
                nc.vector.reciprocal(invk[:], sk[:])
                invbk = gn[:, NKV * T:2 * NKV * T]
                nc.tensor.matmul(invbk[:, 0:512], ones1_r[:], invk[:, 0:512], start=True, stop=True,
                                 skip_group_check=True)
                nc.tensor.matmul(invbk[:, 512:1024], ones1_r[:], invk[:, 512:1024], start=True, stop=True,
                                 skip_group_check=True)
                ktw = rp.tile([128, NKV * T], f32, tag="r4", name="ktw", bufs=4)
                nc.vector.tensor_scalar(ktw[:], kraw[:], knw[:], None, ALU.mult)
                cs1r = bass.AP(cs1.tensor, 0, [[T, 128], [0, NKV], [1, T]])
                cs2r = bass.AP(cs2.tensor, 0, [[T, 128], [0, NKV], [1, T]])
                kA = rp.tile([128, NKV * T], f32, tag="r4", name="kA", bufs=4)
                nc.vector.tensor_tensor(kA[:], ktw[:], cs1r, ALU.mult)
                kB = rp.tile([128, NKV * T], f32, tag="r4", name="kB", bufs=4)
                nc.vector.tensor_tensor(kB[:], ktw[:], cs2r, ALU.mult)
                kR = rp.tile([128, NKV * T], f32, tag="r4", name="kR", bufs=4)
                kAh = rp.tile([64, NKV * T], f32, tag="rh", name="kAh", bufs=2)
                nc.vector.tensor_copy(kAh[:], kA[64:128, :])
                nc.vector.tensor_tensor(kR[0:64, :], kA[0:64, :], kAh[:], ALU.subtract)
                kBh = rp.tile([64, NKV * T], f32, tag="rh", name="kBh", bufs=2)
                nc.vector.tensor_copy(kBh[:], kB[64:128, :])
                nc.vector.tensor_tensor(kR[64:128, :], kBh[:], kB[0:64, :], ALU.add)
                nc.vector.tensor_tensor(kT_c[:], kR[:], invbk, ALU.mult)
                for j in range(NKV):
                    nc.sync.dma_start(cc_in_k[128 * j:128 * (j + 1), :], kT_c[:, T * j:T * (j + 1)])

                # V projection: token-major [T, 512]
                v_sb = pre.tile([128, 2 * NKV * HD], bf16)
                gV = psum.tile([128, 2048], f32, tag="g", name="gV")
                for i in range(DT):
                    wv = wpool.tile([128, 2048], f32r, tag="w", name=f"wv{i}")
                    nc.sync.dma_start(wv[:, 0:NKV * HD], wvT_d[128 * i:128 * (i + 1), :])
                    for h2 in range(2):
                        nc.tensor.matmul(gV[:, 512 * h2:512 * (h2 + 1)],
                                         xnT[:, T * i + 128 * h2:T * i + 128 * h2 + 128],
                                         wv[:, 0:NKV * HD],
                                         start=(i == 0), stop=(i == DT - 1), skip_group_check=True)
                for h2 in range(2):
                    nc.vector.tensor_copy(v_sb[:, 512 * h2:512 * (h2 + 1)], gV[:, 512 * h2:512 * (h2 + 1)])
                    nc.sync.dma_start(cc_in_v[128 * h2:128 * (h2 + 1), :], v_sb[:, 512 * h2:512 * (h2 + 1)])

                # launch the all-gathers
                nc.gpsimd.collective_compute(
                    "AllGather", ALU.bypass, replica_groups=rg,
                    ins=[cc_in_k[:].opt()], outs=[cc_out_k[:].opt()])
                nc.gpsimd.collective_compute(
                    "AllGather", ALU.bypass, replica_groups=rg,
                    ins=[cc_in_v[:].opt()], outs=[cc_out_v[:].opt()])
                if DEBUG:
                    for j in range(NKV):
                        nc.sync.dma_start(dbg_k[128 * j:128 * (j + 1), :], kT_c[:, T * j:T * (j + 1)].bitcast(f32))
                    vdbg = pre.tile([128, 2 * NKV * HD], f32)
                    for h2 in range(2):
                        nc.vector.tensor_copy(vdbg[:, 512 * h2:512 * (h2 + 1)], v_sb[:, 512 * h2:512 * (h2 + 1)])
                        nc.sync.dma_start(dbg_v[128 * h2:128 * (h2 + 1), :], vdbg[:, 512 * h2:512 * (h2 + 1)])

                # ================= phase A: routed Q projection (overlaps AG) ====
                for half in range(2):
                    gQ = psum.tile([128, 2048], f32, tag="g", name=f"gQ{half}")
                    for i in range(DT):
                        wqt = wpool.tile([128, 2048], f32r, tag="w", name=f"wq{half}_{i}")
                        nc.sync.dma_start(wqt[:, 0:1024], wq_d[128 * i:128 * (i + 1), 1024 * half:1024 * (half + 1)])
                        for jj in range(8):
                            mm = nc.tensor.matmul(gQ[:, T * jj:T * (jj + 1)],
                                                  wqt[:, 128 * jj:128 * (jj + 1)],
                                                  xnT[:, T * i:T * (i + 1)],
                                                  start=(i == 0 and jj % 2 == 0), stop=(i == DT - 1),
                                                  skip_group_check=True)
                            if i == 0:
                                if jj % 2 == 0:
                                    bank_start = mm
                                else:
                                    add_dep_helper(mm.ins, bank_start.ins, sync=False,
                                                   reason="psum bank pair order")
                    # drain + q rmsnorm + rope, 4 heads at a time
                    for g4 in range(2):
                        jb = 8 * half + 4 * g4
                        q4 = sqp.tile([128, 4 * T], f32, tag="qraw", name=f"qraw{jb}", bufs=2)
                        nc.vector.tensor_copy(q4[:], gQ[:, 4 * T * g4:4 * T * (g4 + 1)])
                        qsq = sqp.tile([128, 4 * T], f32r, tag="sq4", name=f"qsq{jb}", bufs=2)
                        nc.vector.tensor_tensor(qsq[:], q4[:], q4[:], ALU.mult)
                        gn = psum.tile([128, 2048], f32, tag="g", name=f"gqn{jb}")
                        ssq = gn[0:1, 0:4 * T]
                        nc.tensor.matmul(gn[0:1, 0:512], ones_r[:], qsq[:, 0:512], start=True, stop=True,
                                         skip_group_check=True)
                        nc.tensor.matmul(gn[0:1, 512:1024], ones_r[:], qsq[:, 512:1024], start=True, stop=True,
                                         skip_group_check=True)
                        # 1/sqrt(ss + 128*eps) = rms(q)^-1 / sqrt(HD): folds in the score scale
                        sq4 = vec.tile([1, 4 * T], f32, tag="s4", name=f"sq4_{jb}")
                        nc.scalar.activation(sq4[:], ssq, AF.Sqrt, bias=epsB[:], scale=1.0)
                        invq = vec.tile([1, 4 * T], f32r, tag="inv4", name=f"invq{jb}")
                        nc.vector.reciprocal(invq[:], sq4[:])
                        invbq = gn[:, 4 * T:8 * T]
                        nc.tensor.matmul(invbq[:, 0:512], ones1_r[:], invq[:, 0:512], start=True, stop=True,
                                         skip_group_check=True)
                        nc.tensor.matmul(invbq[:, 512:1024], ones1_r[:], invq[:, 512:1024], start=True, stop=True,
                                         skip_group_check=True)
                        qtw = rp.tile([128, 4 * T], f32, tag="r4", name=f"qtw{jb}", bufs=4)
                        nc.vector.tensor_scalar(qtw[:], q4[:], qnw[:], None, ALU.mult)
                        cs1r = bass.AP(cs1.tensor, 0, [[T, 128], [0, 4], [1, T]])
                        cs2r = bass.AP(cs2.tensor, 0, [[T, 128], [0, 4], [1, T]])
                        qA = rp.tile([128, 4 * T], f32, tag="r4", name=f"qA{jb}", bufs=4)
                        nc.vector.tensor_tensor(qA[:], qtw[:], cs1r, ALU.mult)
                        qB = rp.tile([128, 4 * T], f32, tag="r4", name=f"qB{jb}", bufs=4)
                        nc.vector.tensor_tensor(qB[:], qtw[:], cs2r, ALU.mult)
                        qR = rp.tile([128, 4 * T], f32, tag="r4", name=f"qR{jb}", bufs=4)
                        qAh = rp.tile([64, 4 * T], f32, tag="rh", name=f"qAh{jb}", bufs=2)
                        nc.vector.tensor_copy(qAh[:], qA[64:128, :])
                        nc.vector.tensor_tensor(qR[0:64, :], qA[0:64, :], qAh[:], ALU.subtract)
                        qBh = rp.tile([64, 4 * T], f32, tag="rh", name=f"qBh{jb}", bufs=2)
                        nc.vector.tensor_copy(qBh[:], qB[64:128, :])
                        nc.vector.tensor_tensor(qR[64:128, :], qBh[:], qB[0:64, :], ALU.add)
                        nc.vector.tensor_tensor(qT[:, T * jb:T * (jb + 4)], qR[:], invbq, ALU.mult)

            if DEBUG:
                for j in range(NH):
                    nc.sync.dma_start(dbg_q[128 * j:128 * (j + 1), :], qT[:, T * j:T * (j + 1)].bitcast(f32))

            # ================= attention =================
            with tc.tile_pool(name="att", bufs=1) as att, \
                    tc.tile_pool(name="kst", bufs=6) as kstp, \
                    tc.tile_pool(name="vst", bufs=6) as vstp, \
                    tc.tile_pool(name="exp", bufs=3) as expp:
                mask_sb = att.tile([128, 16 * T], bf16)
                for t in range(16):
                    nc.sync.dma_start(mask_sb[:, T * t:T * (t + 1)], mask_d[128 * t:128 * (t + 1), :])

                for g in range(NKV):
                    gOut = psum.tile([128, 2048], f32, tag="g", name=f"gOut{g}")
                    outP = gOut[:, 0:4 * T]
                    denP = gOut[0:1, 4 * T:8 * T]  # [1, 1024] in banks 2-3 (disjoint from outP)
                    kch = []
                    for r in range(NC):
                        kc = kstp.tile([128, T], f32r, tag="k", name=f"kch{g}_{r}")
                        nc.sync.dma_start(kc[:], cc_out_k[512 * r + 128 * g:512 * r + 128 * (g + 1), :])
                        kch.append(kc)
                    gSc = psum.tile([128, 2048], f32, tag="g", name=f"gSc{g}")
                    for kt in range(16):
                        r, half = kt // 2, kt % 2
                        ksl = kch[r][:, 128 * half:128 * (half + 1)]
                        vt = vstp.tile([128, 128], bf16, tag="v", name=f"vt{g}_{kt}")
                        nc.sync.dma_start(vt[:], cc_out_v[128 * kt:128 * (kt + 1), 128 * g:128 * (g + 1)])
                        sc = gSc[:, 1024 * (kt % 2):1024 * (kt % 2) + 1024]
                        for h in range(4):
                            nc.tensor.matmul(sc[:, T * h:T * (h + 1)], ksl,
                                             qT[:, T * (4 * g + h):T * (4 * g + h + 1)],
                                             start=True, stop=True, skip_group_check=True)
                        ex = expp.tile([128, 4 * T], bf16, tag="e", name=f"ex{g}_{kt}")
                        nc.scalar.activation(ex[:], sc, AF.Exp, bias=0.0, scale=1.0)
                        mrep = bass.AP(mask_sb.tensor, T * kt, [[16 * T, 128], [0, 4], [1, T]])
                        nc.vector.tensor_tensor(ex[:], ex[:], mrep, ALU.mult)
                        for h in range(4):
                            mm = nc.tensor.matmul(outP[:, T * h:T * (h + 1)], vt[:], ex[:, T * h:T * (h + 1)],
                                                  start=(kt == 0 and h % 2 == 0), stop=(kt == 15),
                                                  skip_group_check=True)
                            if kt == 0:
                                if h % 2 == 0:
                                    bank_start_o = mm
                                else:
                                    add_dep_helper(mm.ins, bank_start_o.ins, sync=False,
                                                   reason="psum bank pair order")
                        for h in range(4):
                            mm = nc.tensor.matmul(denP[0:1, T * h:T * (h + 1)], ones_b[:], ex[:, T * h:T * (h + 1)],
                                                  start=(kt == 0 and h % 2 == 0), stop=(kt == 15),
                                                  skip_group_check=True)
                            if kt == 0:
                                if h % 2 == 0:
                                    bank_start_d = mm
                                else:
                                    add_dep_helper(mm.ins, bank_start_d.ins, sync=False,
                                                   reason="psum bank pair order")
                    rec = vec.tile([1, 4 * T], f32r, tag="rec", name=f"rec{g}")
                    nc.vector.reciprocal(rec[:], denP)
                    bc = gSc[:, 0:4 * T]
                    nc.tensor.matmul(bc[:, 0:512], ones1_r[:], rec[:, 0:512], start=True, stop=True,
                                     skip_group_check=True)
                    nc.tensor.matmul(bc[:, 512:1024], ones1_r[:], rec[:, 512:1024], start=True, stop=True,
                                     skip_group_check=True)
                    # DVE can read only one PSUM operand: stage the broadcast in SBUF
                    bcS = vec.tile([128, 4 * T], bf16, tag="bcS", name=f"bcS{g}")
                    nc.vector.tensor_copy(bcS[:], bc)
                    nc.vector.tensor_tensor(attnT[:, 4 * T * g:4 * T * (g + 1)], outP, bcS[:], ALU.mult)

            # ================= o proj + residual =================
            gO1 = psum.tile([128, 2048], f32, tag="g", name="gO1")
            gO2 = psum.tile([128, 2048], f32, tag="g", name="gO2")
            for i in range(DT):
                wot = wpool.tile([128, 2048], f32r, tag="w", name=f"wo{i}")
                nc.sync.dma_start(wot[:], wo_d[128 * i:128 * (i + 1), :])
                for j in range(DT):
                    gdst = gO1 if j < 8 else gO2
                    mm = nc.tensor.matmul(gdst[:, T * (j % 8):T * (j % 8 + 1)],
                                          wot[:, 128 * j:128 * (j + 1)],
                                          attnT[:, T * i:T * (i + 1)],
                                          start=(i == 0 and j % 2 == 0), stop=(i == DT - 1),
                                          skip_group_check=True)
                    if i == 0:
                        if j % 2 == 0:
                            bank_start = mm
                        else:
                            add_dep_helper(mm.ins, bank_start.ins, sync=False,
                                           reason="psum bank pair order")
            for j in range(DT):
                gdst = gO1 if j < 8 else gO2
                nc.vector.tensor_tensor(h1T[:, T * j:T * (j + 1)], gdst[:, T * (j % 8):T * (j % 8 + 1)],
                                        xT[:, T * j:T * (j + 1)], ALU.add)

            if DEBUG:
                for j in range(NH):
                    nc.sync.dma_start(dbg_at[128 * j:128 * (j + 1), :], attnT[:, T * j:T * (j + 1)].bitcast(f32))
                for j in range(DT):
                    nc.sync.dma_start(dbg_h1[128 * j:128 * (j + 1), :], h1T[:, T * j:T * (j + 1)].bitcast(f32))

            # ================= mu guidance =================
            gM1 = psum.tile([128, 2048], f32, tag="g", name="gM1")
            gM2 = psum.tile([128, 2048], f32, tag="g", name="gM2")
            for i in range(DT):
                wmt = wpool.tile([128, 2048], f32r, tag="w", name=f"wmu{i}")
                nc.sync.dma_start(wmt[:], wmuT_d[128 * i:128 * (i + 1), :])
                for j in range(DT):
                    gdst = gM1 if j < 8 else gM2
                    mm = nc.tensor.matmul(gdst[:, T * (j % 8):T * (j % 8 + 1)],
                                          wmt[:, 128 * j:128 * (j + 1)],
                                          h1T[:, T * i:T * (i + 1)],
                                          start=(i == 0 and j % 2 == 0), stop=(i == DT - 1),
                                          skip_group_check=True)
                    if i == 0:
                        if j % 2 == 0:
                            bank_start = mm
                        else:
                            add_dep_helper(mm.ins, bank_start.ins, sync=False,
                                           reason="psum bank pair order")
            for j in range(DT):
                gdst = gM1 if j < 8 else gM2
                mu_sb = sqp.tile([128, T], f32, tag="mu", name=f"mu{j}", bufs=2)
                nc.vector.tensor_scalar(mu_sb[:], gdst[:, T * (j % 8):T * (j % 8 + 1)],
                                        muclip[:, j:j + 1], None, ALU.add)
                nc.sync.dma_start(muout_d[128 * j:128 * (j + 1), :], mu_sb[:])

            # ================= post norm + MLP =================
            with tc.tile_pool(name="mlp", bufs=1) as mlp:
                gP = psum.tile([128, 2048], f32, tag="g", name="gP")
                ssP = gP[0:1, 0:T]
                for i in range(DT):
                    hsq = sqp.tile([128, T], f32r, tag="sq", name=f"hsq{i}")
                    nc.vector.tensor_tensor(hsq[:], h1T[:, T * i:T * (i + 1)].bitcast(f32),
                                            h1T[:, T * i:T * (i + 1)].bitcast(f32), ALU.mult)
                    nc.tensor.matmul(ssP, ones_r[:], hsq[:], start=(i == 0), stop=(i == DT - 1),
                                     skip_group_check=True)
                invP = rnorm_vec(ssP, epsA, 1.0 / D)
                invbP = gP[:, T:2 * T]
                nc.tensor.matmul(invbP, ones1_r[:], invP[:], start=True, stop=True,
                                 skip_group_check=True)
                h2T = mlp.tile([128, DT * T], f32r)
                for i in range(DT):
                    nc.vector.scalar_tensor_tensor(
                        h2T[:, T * i:T * (i + 1)], h1T[:, T * i:T * (i + 1)].bitcast(f32),
                        lnpost[:, i:i + 1], invbP, ALU.mult, ALU.mult)

                # gate/up projection in 2 passes of 8 EI-tiles
                actT = mlp.tile([128, DT * T], f32r)
                for p in range(2):
                    gG = psum.tile([128, 2048], f32, tag="g", name=f"gG{p}")
                    gU = psum.tile([128, 2048], f32, tag="g", name=f"gU{p}")
                    for i in range(DT):
                        wgt = wpool.tile([128, 2048], f32r, tag="w", name=f"wgu{p}_{i}")
                        nc.sync.dma_start(wgt[:, 0:1024],
                                          wgu_d[128 * i:128 * (i + 1), 1024 * p:1024 * (p + 1)])
                        nc.sync.dma_start(wgt[:, 1024:2048],
                                          wgu_d[128 * i:128 * (i + 1), EI + 1024 * p:EI + 1024 * (p + 1)])
                        for jj in range(8):
                            mmg = nc.tensor.matmul(gG[:, T * jj:T * (jj + 1)],
                                                   wgt[:, 128 * jj:128 * (jj + 1)],
                                                   h2T[:, T * i:T * (i + 1)],
                                                   start=(i == 0 and jj % 2 == 0), stop=(i == DT - 1),
                                                   skip_group_check=True)
                            mmu = nc.tensor.matmul(gU[:, T * jj:T * (jj + 1)],
                                                   wgt[:, 1024 + 128 * jj:1024 + 128 * (jj + 1)],
                                                   h2T[:, T * i:T * (i + 1)],
                                                   start=(i == 0 and jj % 2 == 0), stop=(i == DT - 1),
                                                   skip_group_check=True)
                            if i == 0:
                                if jj % 2 == 0:
                                    bank_start_g, bank_start_u = mmg, mmu
                                else:
                                    add_dep_helper(mmg.ins, bank_start_g.ins, sync=False,
                                                   reason="psum bank pair order")
                                    add_dep_helper(mmu.ins, bank_start_u.ins, sync=False,
                                                   reason="psum bank pair order")
                    for jj in range(8):
                        sg = sqp.tile([128, T], f32, tag="sg", name=f"sg{p}_{jj}", bufs=2)
                        nc.scalar.activation(sg[:], gG[:, T * jj:T * (jj + 1)], AF.Sigmoid, bias=0.0, scale=1.0)
                        sx = sqp.tile([128, T], f32, tag="sx", name=f"sx{p}_{jj}", bufs=2)
                        nc.vector.tensor_tensor(sx[:], sg[:], gG[:, T * jj:T * (jj + 1)], ALU.mult)
                        nc.vector.tensor_tensor(actT[:, T * (8 * p + jj):T * (8 * p + jj + 1)],
                                                sx[:], gU[:, T * jj:T * (jj + 1)], ALU.mult)

                # down projection + final residual
                gD1 = psum.tile([128, 2048], f32, tag="g", name="gD1")
                gD2 = psum.tile([128, 2048], f32, tag="g", name="gD2")
                for i in range(DT):
                    wdt = wpool.tile([128, 2048], f32r, tag="w", name=f"wdw{i}")
                    nc.sync.dma_start(wdt[:], wdw_d[128 * i:128 * (i + 1), :])
                    for j in range(DT):
                        gdst = gD1 if j < 8 else gD2
                        mm = nc.tensor.matmul(gdst[:, T * (j % 8):T * (j % 8 + 1)],
                                              wdt[:, 128 * j:128 * (j + 1)],
                                              actT[:, T * i:T * (i + 1)],
                                              start=(i == 0 and j % 2 == 0), stop=(i == DT - 1),
                                              skip_group_check=True)
                        if i == 0:
                            if j % 2 == 0:
                                bank_start = mm
                            else:
                                add_dep_helper(mm.ins, bank_start.ins, sync=False,
                                               reason="psum bank pair order")
                for j in range(DT):
                    gdst = gD1 if j < 8 else gD2
                    ho = sqp.tile([128, T], f32, tag="mu", name=f"ho{j}", bufs=2)
                    nc.vector.tensor_tensor(ho[:], gdst[:, T * (j % 8):T * (j % 8 + 1)],
                                            h1T[:, T * j:T * (j + 1)].bitcast(f32), ALU.add)
                    nc.sync.dma_start(hout_d[128 * j:128 * (j + 1), :], ho[:])

    nc.compile()
    return nc


def make_in_maps(inputs):
    hs = np.asarray(inputs["hidden_states"], np.float32)
    positions = np.asarray(inputs["positions"]).astype(np.int64)
    sort_idx = np.asarray(inputs["sort_idx"]).astype(np.int64)
    ln_in = np.asarray(inputs["input_ln_w"], np.float32)
    q_proj_w = np.asarray(inputs["q_proj_w"], np.float32)
    o_proj_w = np.asarray(inputs["o_proj_w"], np.float32)
    k_w = np.asarray(inputs["k_w"], np.float32)
    v_w = np.asarray(inputs["v_w"], np.float32)
    q_norm_w = np.asarray(inputs["q_norm_w"], np.float32)
    k_norm_w = np.asarray(inputs["k_norm_w"], np.float32)
    mu = np.asarray(inputs["mu"], np.float32)
    mu_proj_w = np.asarray(inputs["mu_proj_w"], np.float32)
    post_ln = np.asarray(inputs["post_ln_w"], np.float32)
    gate_up = np.asarray(inputs["gate_up_proj"], np.float32)
    down = np.asarray(inputs["down_proj"], np.float32)

    pos_sorted = positions[sort_idx]
    inv_freq = (1.0 / (np.float32(THETA) ** (np.arange(0, 64, dtype=np.float32) / np.float32(64)))).astype(np.float32)
    wkT = np.ascontiguousarray(k_w.T)
    wvT = np.ascontiguousarray(v_w.T)
    wmuT = np.ascontiguousarray(mu_proj_w.T)
    lnin_r = np.ascontiguousarray(ln_in.reshape(DT, 128).T)
    lnpost_r = np.ascontiguousarray(post_ln.reshape(DT, 128).T)
    mu_r = np.ascontiguousarray(mu.reshape(DT, 128).T)
    qnw_r = np.ascontiguousarray(q_norm_w.reshape(128, 1))
    knw_r = np.ascontiguousarray(k_norm_w.reshape(128, 1))

    in_maps = []
    for c in range(NC):
        S = sort_idx[T * c:T * (c + 1)]
        e = c // 2
        pos_c = positions[S].astype(np.float32)
        ang = pos_c[None, :] * inv_freq[:, None]
        mask = (pos_sorted[:, None] <= positions[S][None, :])
        in_maps.append({
            "xT": np.ascontiguousarray(hs[S].T),
            "cosT": np.cos(ang).astype(np.float32),
            "sinT": np.sin(ang).astype(np.float32),
            "maskb": mask.astype(ml_dtypes.bfloat16),
            "lnw_in": lnin_r,
            "lnw_post": lnpost_r,
            "qnw": qnw_r,
            "knw": knw_r,
            "muvec": mu_r,
            "wq": np.ascontiguousarray(q_proj_w[e]),
            "wo": np.ascontiguousarray(o_proj_w[e]),
            "wgu": np.ascontiguousarray(gate_up[e]),
            "wdw": np.ascontiguousarray(down[e]),
            "wmuT": wmuT,
            "wkT": wkT,
            "wvT": wvT,
        })
    return in_maps, sort_idx


def assemble(results, sort_idx):
    h_full = np.empty((N, D), np.float32)
    mu_full = np.empty((N, D), np.float32)
    for c in range(NC):
        S = sort_idx[T * c:T * (c + 1)]
        h_full[S] = results[c]["h_out"].T
        mu_full[S] = results[c]["mu_out"].T
    return h_full, mu_full


def kernel(**inputs):
    if "nc" not in _cached:
        _cached["nc"] = _build()
    nc = _cached["nc"]
    in_maps, sort_idx = make_in_maps(inputs)
    res = run_bass_kernel_spmd(nc, in_maps, core_ids=list(range(NC)))
    return assemble(res.results, sort_idx)


# revision 20
# speedup vs baseline: 1.0352x; 1.0352x over previous
"""Expert-parallel + token-parallel Trainium2 kernel for ComplexityDecoderLayerV2.

Distribution (8 cores, E=4 experts):
  - Tokens are pre-sorted by expert on the host (sort_idx); core c owns the
    256 sorted tokens S_c = sort_idx[256c:256c+256] (expert e = c//2) and all
    per-token compute for them: input rmsnorm, routed q projection, its slice
    of K/V, all 16 attention heads for its 256 query tokens, routed o
    projection, residuals, mu projection, post norm, and the routed MLP.
  - K/V are computed per-core for the core's tokens, then AllGather'd
    (replacing the reference's gather/scatter with a collective, per the
    sharding hint). Causality over the shuffled token order is enforced with a
    host-built multiplicative {0,1} mask on the scores.
  - Activations are kept feature-major ([feature, token]) on-chip so every
    matmul chains without transposes; host transposes/un-permutes at the
    boundary (pure data movement).
Compute dtype: float32r matmuls (fp32 storage, ~1e-4 matmul rel err) with a
bf16 softmax/V path; all accumulation in fp32 PSUM.
"""

import sys

if "/opt/trn_rl_repo" not in sys.path:
    sys.path.insert(0, "/opt/trn_rl_repo")

import numpy as np
import ml_dtypes

import concourse.bass as bass
import concourse.mybir as mybir
import concourse.tile as tile
from concourse import bacc
from concourse.bass_utils import run_bass_kernel_spmd
from concourse.tile import add_dep_helper

f32 = mybir.dt.float32
f32r = mybir.dt.float32r
bf16 = mybir.dt.bfloat16
AF = mybir.ActivationFunctionType
ALU = mybir.AluOpType

NC = 8            # cores
N = 2048          # tokens
D = 2048          # hidden
NH = 16           # q heads
NKV = 4           # kv heads
HD = 128          # head dim
E = 4             # experts
EI = 2048         # per-expert intermediate
T = N // NC       # tokens per core = 256
DT = D // 128     # 16 feature tiles
EPS = 1e-6
THETA = 10000.0

_cached = {}


def _build():
    nc = bacc.Bacc("TRN2", target_bir_lowering=False, debug=False, num_devices=NC)

    # ---- per-core external inputs ----
    xT_d = nc.dram_tensor("xT", [D, T], f32, kind="ExternalInput")
    cos_d = nc.dram_tensor("cosT", [64, T], f32, kind="ExternalInput")
    sin_d = nc.dram_tensor("sinT", [64, T], f32, kind="ExternalInput")
    mask_d = nc.dram_tensor("maskb", [N, T], bf16, kind="ExternalInput")
    lnin_d = nc.dram_tensor("lnw_in", [128, DT], f32, kind="ExternalInput")
    lnpost_d = nc.dram_tensor("lnw_post", [128, DT], f32, kind="ExternalInput")
    qnw_d = nc.dram_tensor("qnw", [128, 1], f32, kind="ExternalInput")
    knw_d = nc.dram_tensor("knw", [128, 1], f32, kind="ExternalInput")
    mu_d = nc.dram_tensor("muvec", [128, DT], f32, kind="ExternalInput")
    wq_d = nc.dram_tensor("wq", [D, NH * HD], f32r, kind="ExternalInput")
    wo_d = nc.dram_tensor("wo", [NH * HD, D], f32r, kind="ExternalInput")
    wgu_d = nc.dram_tensor("wgu", [D, 2 * EI], f32r, kind="ExternalInput")
    wdw_d = nc.dram_tensor("wdw", [EI, D], f32r, kind="ExternalInput")
    wmuT_d = nc.dram_tensor("wmuT", [D, D], f32r, kind="ExternalInput")
    wkT_d = nc.dram_tensor("wkT", [D, NKV * HD], f32r, kind="ExternalInput")
    wvT_d = nc.dram_tensor("wvT", [D, NKV * HD], f32r, kind="ExternalInput")
    hout_d = nc.dram_tensor("h_out", [D, T], f32, kind="ExternalOutput")
    muout_d = nc.dram_tensor("mu_out", [D, T], f32, kind="ExternalOutput")
    DEBUG = bool(__import__("os").environ.get("KERNEL_DEBUG"))
    if DEBUG:
        dbg_xn = nc.dram_tensor("dbg_xn", [D, T], f32, kind="ExternalOutput")
        dbg_q = nc.dram_tensor("dbg_q", [D, T], f32, kind="ExternalOutput")
        dbg_at = nc.dram_tensor("dbg_at", [D, T], f32, kind="ExternalOutput")
        dbg_h1 = nc.dram_tensor("dbg_h1", [D, T], f32, kind="ExternalOutput")
        dbg_k = nc.dram_tensor("dbg_k", [NKV * HD, T], f32, kind="ExternalOutput")
        dbg_v = nc.dram_tensor("dbg_v", [T, NKV * HD], f32, kind="ExternalOutput")

    from contextlib import ExitStack

    with tile.TileContext(nc) as tc, \
            nc.allow_low_precision(reason="f32r/bf16 compute by design"), \
            ExitStack() as stk:
        rg = [list(range(NC))]

        perm = stk.enter_context(tc.tile_pool(name="perm", bufs=1))
        wpool = stk.enter_context(tc.tile_pool(name="wpool", bufs=6))
        sqp = stk.enter_context(tc.tile_pool(name="sqp", bufs=2))
        rp = stk.enter_context(tc.tile_pool(name="rp", bufs=8))
        vec = stk.enter_context(tc.tile_pool(name="vec", bufs=2))
        psum = stk.enter_context(tc.tile_pool(name="psum", bufs=2, space="PSUM"))
        dram = stk.enter_context(tc.tile_pool(name="dram", bufs=1, space="DRAM"))

        if True:
            # ---- constants ----
            epsA = perm.tile([1, 1], f32)
            nc.vector.memset(epsA[:], EPS)
            epsB = perm.tile([1, 1], f32)
            nc.vector.memset(epsB[:], 128.0 * EPS)
            ones_f = perm.tile([128, 1], f32)
            nc.vector.memset(ones_f[:], 1.0)
            ones_r = perm.tile([128, 1], f32r)
            nc.vector.tensor_copy(ones_r[:], ones_f[:])
            ones1_f = perm.tile([1, 128], f32)
            nc.vector.memset(ones1_f[:], 1.0)
            ones1_r = perm.tile([1, 128], f32r)
            nc.vector.tensor_copy(ones1_r[:], ones1_f[:])
            ones_b = perm.tile([128, 1], bf16)
            nc.vector.memset(ones_b[:], 1.0)
            lnin = perm.tile([128, DT], f32)
            nc.sync.dma_start(lnin[:], lnin_d[:])
            lnpost = perm.tile([128, DT], f32)
            nc.sync.dma_start(lnpost[:], lnpost_d[:])
            qnw = perm.tile([128, 1], f32)
            nc.sync.dma_start(qnw[:], qnw_d[:])
            knw = perm.tile([128, 1], f32)
            nc.sync.dma_start(knw[:], knw_d[:])
            muv = perm.tile([128, DT], f32)
            nc.sync.dma_start(muv[:], mu_d[:])
            muclip = perm.tile([128, DT], f32)
            nc.vector.tensor_scalar(muclip[:], muv[:], 0.0, 2.0, ALU.max, ALU.min)
            cosT = perm.tile([64, T], f32)
            nc.sync.dma_start(cosT[:], cos_d[:])
            sinT = perm.tile([64, T], f32)
            nc.sync.dma_start(sinT[:], sin_d[:])
            cs1 = perm.tile([128, T], f32)
            nc.vector.tensor_copy(cs1[0:64, :], cosT[:])
            nc.vector.tensor_copy(cs1[64:128, :], sinT[:])
            cs2 = perm.tile([128, T], f32)
            nc.vector.tensor_copy(cs2[0:64, :], sinT[:])
            nc.vector.tensor_copy(cs2[64:128, :], cosT[:])

            # ---- persistent activations ----
            xT = perm.tile([128, DT * T], f32)
            for i in range(DT):
                nc.sync.dma_start(xT[:, T * i:T * (i + 1)], xT_d[128 * i:128 * (i + 1), :])
            qT = perm.tile([128, NH * T], f32r)
            attnT = perm.tile([128, NH * T], f32r)
            h1T = perm.tile([128, DT * T], f32r)

            # collective bounce buffers
            cc_in_k = dram.tile([NKV * HD, T], f32r)
            cc_in_v = dram.tile([T, NKV * HD], bf16)
            cc_out_k = dram.tile([NC * NKV * HD, T], f32r, addr_space="Shared")
            cc_out_v = dram.tile([N, NKV * HD], bf16, addr_space="Shared")

            def rnorm_vec(ss_ap, bias_t, scale):
                """1/sqrt(scale*ss + bias) as a [1, T] f32r vector + [128, T] psum bcast."""
                s_t = vec.tile([1, T], f32, tag="s", name=f"s_{nc.next_id()}")
                nc.scalar.activation(s_t[:], ss_ap, AF.Sqrt, bias=bias_t[:], scale=scale)
                inv_t = vec.tile([1, T], f32r, tag="inv", name=f"inv_{nc.next_id()}")
                nc.vector.reciprocal(inv_t[:], s_t[:])
                return inv_t

            # ================= phase A: input rmsnorm =================
            gA = psum.tile([128, 2048], f32, tag="g", name="gA")
            ssA = gA[0:1, 0:T]
            for i in range(DT):
                xsq = sqp.tile([128, T], f32r, tag="sq", name=f"xsq{i}")
                nc.vector.tensor_tensor(xsq[:], xT[:, T * i:T * (i + 1)], xT[:, T * i:T * (i + 1)], ALU.mult)
                nc.tensor.matmul(ssA, ones_r[:], xsq[:], start=(i == 0), stop=(i == DT - 1),
                                 skip_group_check=True)
            invA = rnorm_vec(ssA, epsA, 1.0 / D)
            invbA = gA[:, T:2 * T]
            nc.tensor.matmul(invbA, ones1_r[:], invA[:], start=True, stop=True,
                             skip_group_check=True)
            with tc.tile_pool(name="pre", bufs=1) as pre:
                xnT = pre.tile([128, DT * T], f32r)
                for i in range(DT):
                    nc.vector.scalar_tensor_tensor(
                        xnT[:, T * i:T * (i + 1)], xT[:, T * i:T * (i + 1)],
                        lnin[:, i:i + 1], invbA, ALU.mult, ALU.mult)

                if DEBUG:
                    for i in range(DT):
                        nc.sync.dma_start(dbg_xn[128 * i:128 * (i + 1), :], xnT[:, T * i:T * (i + 1)].bitcast(f32))

                # ================= phase A: K/V projections (first, to launch AG early) ====
                kraw = pre.tile([128, NKV * T], f32)
                gK = psum.tile([128, 2048], f32, tag="g", name="gK")
                for i in range(DT):
                    wk = wpool.tile([128, 2048], f32r, tag="w", name=f"wk{i}")
                    nc.sync.dma_start(wk[:, 0:NKV * HD], wkT_d[128 * i:128 * (i + 1), :])
                    for j in range(NKV):
                        mm = nc.tensor.matmul(gK[:, T * j:T * (j + 1)], wk[:, 128 * j:128 * (j + 1)],
                                              xnT[:, T * i:T * (i + 1)],
                                              start=(i == 0 and j % 2 == 0), stop=(i == DT - 1),
                                              skip_group_check=True)
                        if i == 0:
                            if j % 2 == 0:
                                bank_start = mm
                            else:
                                add_dep_helper(mm.ins, bank_start.ins, sync=False,
                                               reason="psum bank pair order")
                for j in range(NKV):
                    nc.vector.tensor_copy(kraw[:, T * j:T * (j + 1)], gK[:, T * j:T * (j + 1)])

                # k rmsnorm + rope (all 4 heads batched), then to bounce
                kT_c = pre.tile([128, NKV * T], f32r)
                ksq = sqp.tile([128, NKV * T], f32r, tag="sq4", name="ksq", bufs=2)
                nc.vector.tensor_tensor(ksq[:], kraw[:], kraw[:], ALU.mult)
                gn = psum.tile([128, 2048], f32, tag="g", name="gkn")
                ssk = gn[0:1, 0:NKV * T]
                nc.tensor.matmul(gn[0:1, 0:512], ones_r[:], ksq[:, 0:512], start=True, stop=True,
                                 skip_group_check=True)
                nc.tensor.matmul(gn[0:1, 512:1024], ones_r[:], ksq[:, 512:1024], start=True, stop=True,
                                 skip_group_check=True)
                sk = vec.tile([1, NKV * T], f32, tag="s4", name="sk")
                nc.scalar.activation(sk[:], ssk, AF.Sqrt, bias=epsA[:], scale=1.0 / HD)
                invk = vec.tile([1, NKV * T], f32r, tag="inv4", name="invk")
                nc.vector.reciprocal(invk[:], sk[:])
                invbk = gn[:, NKV * T:2 * NKV * T]
                nc.tensor.matmul(invbk[:, 0:512], ones1_r[:], invk[:, 0:512], start=True, stop=True,
                                 skip_group_check=True)
                nc.tensor.matmul(invbk[:, 512:1024], ones1_r[:], invk[:, 512:1024], start=True, stop=True,
                                 skip_group_check=True)
                ktw = rp.tile([128, NKV * T], f32, tag="r4", name="ktw", bufs=4)
                nc.vector.tensor_scalar(ktw[:], kraw[:], knw[:], None, ALU.mult)
                cs1r = bass.AP(cs1.tensor, 0, [[T, 128], [0, NKV], [1, T]])
                cs2r = bass.AP(cs2.tensor, 0, [[T, 128], [0, NKV], [1, T]])
                kA = rp.tile([128, NKV * T], f32, tag="r4", name="kA", bufs=4)
                nc.vector.tensor_tensor(kA[:], ktw[:], cs1r, ALU.mult)
                kB = rp.tile([128, NKV * T], f32, tag="r4", name="kB", bufs=4)
                nc.vector.tensor_tensor(kB[:], ktw[:], cs2r, ALU.mult)
                kR = rp.tile([128, NKV * T], f32, tag="r4", name="kR", bufs=4)
                kAh = rp.tile([64, NKV * T], f32, tag="rh", name="kAh", bufs=2)
                nc.vector.tensor_copy(kAh[:], kA[64:128, :])
                nc.vector.tensor_tensor(kR[0:64, :], kA[0:64, :], kAh[:], ALU.subtract)
                kBh = rp.tile([64, NKV * T], f32, tag="rh", name="kBh", bufs=2)
                nc.vector.tensor_copy(kBh[:], kB[64:128, :])
                nc.vector.tensor_tensor(kR[64:128, :], kBh[:], kB[0:64, :], ALU.add)
                nc.vector.tensor_tensor(kT_c[:], kR[:], invbk, ALU.mult)
                for j in range(NKV):
                    nc.sync.dma_start(cc_in_k[128 * j:128 * (j + 1), :], kT_c[:, T * j:T * (j + 1)])

                # V projection: token-major [T, 512]
                v_sb = pre.tile([128, 2 * NKV * HD], bf16)
                gV = psum.tile([128, 2048], f32, tag="g", name="gV")
                for i in range(DT):
                    wv = wpool.tile([128, 2048], f32r, tag="w", name=f"wv{i}")
                    nc.sync.dma_start(wv[:, 0:NKV * HD], wvT_d[128 * i:128 * (i + 1), :])
                    for h2 in range(2):
                        nc.tensor.matmul(gV[:, 512 * h2:512 * (h2 + 1)],
                                         xnT[:, T * i + 128 * h2:T * i + 128 * h2 + 128],
                                         wv[:, 0:NKV * HD],
                                         start=(i == 0), stop=(i == DT - 1), skip_group_check=True)
                for h2 in range(2):
                    nc.vector.tensor_copy(v_sb[:, 512 * h2:512 * (h2 + 1)], gV[:, 512 * h2:512 * (h2 + 1)])
                    nc.sync.dma_start(cc_in_v[128 * h2:128 * (h2 + 1), :], v_sb[:, 512 * h2:512 * (h2 + 1)])

                # launch the all-gathers
                nc.gpsimd.collective_compute(
                    "AllGather", ALU.bypass, replica_groups=rg,
                    ins=[cc_in_k[:].opt()], outs=[cc_out_k[:].opt()])
                nc.gpsimd.collective_compute(
                    "AllGather", ALU.bypass, replica_groups=rg,
                    ins=[cc_in_v[:].opt()], outs=[cc_out_v[:].opt()])
                if DEBUG:
                    for j in range(NKV):
                        nc.sync.dma_start(dbg_k[128 * j:128 * (j + 1), :], kT_c[:, T * j:T * (j + 1)].bitcast(f32))
                    vdbg = pre.tile([128, 2 * NKV * HD], f32)
                    for h2 in range(2):
                        nc.vector.tensor_copy(vdbg[:, 512 * h2:512 * (h2 + 1)], v_sb[:, 512 * h2:512 * (h2 + 1)])
                        nc.sync.dma_start(dbg_v[128 * h2:128 * (h2 + 1), :], vdbg[:, 512 * h2:512 * (h2 + 1)])

                # ================= phase A: routed Q projection (overlaps AG) ====
                for half in range(2):
                    gQ = psum.tile([128, 2048], f32, tag="g", name=f"gQ{half}")
                    for i in range(DT):
                        wqt = wpool.tile([128, 2048], f32r, tag="w", name=f"wq{half}_{i}")
                        nc.sync.dma_start(wqt[:, 0:1024], wq_d[128 * i:128 * (i + 1), 1024 * half:1024 * (half + 1)])
                        for jj in range(8):
                            mm = nc.tensor.matmul(gQ[:, T * jj:T * (jj + 1)],
                                                  wqt[:, 128 * jj:128 * (jj + 1)],
                                                  xnT[:, T * i:T * (i + 1)],
                                                  start=(i == 0 and jj % 2 == 0), stop=(i == DT - 1),
                                                  skip_group_check=True)
                            if i == 0:
                                if jj % 2 == 0:
                                    bank_start = mm
                                else:
                                    add_dep_helper(mm.ins, bank_start.ins, sync=False,
                                                   reason="psum bank pair order")
                    # drain + q rmsnorm + rope, 4 heads at a time
                    for g4 in range(2):
                        jb = 8 * half + 4 * g4
                        q4 = sqp.tile([128, 4 * T], f32, tag="qraw", name=f"qraw{jb}", bufs=2)
                        nc.vector.tensor_copy(q4[:], gQ[:, 4 * T * g4:4 * T * (g4 + 1)])
                        qsq = sqp.tile([128, 4 * T], f32r, tag="sq4", name=f"qsq{jb}", bufs=2)
                        nc.vector.tensor_tensor(qsq[:], q4[:], q4[:], ALU.mult)
                        gn = psum.tile([128, 2048], f32, tag="g", name=f"gqn{jb}")
                        ssq = gn[0:1, 0:4 * T]
                        nc.tensor.matmul(gn[0:1, 0:512], ones_r[:], qsq[:, 0:512], start=True, stop=True,
                                         skip_group_check=True)
                        nc.tensor.matmul(gn[0:1, 512:1024], ones_r[:], qsq[:, 512:1024], start=True, stop=True,
                                         skip_group_check=True)
                        # 1/sqrt(ss + 128*eps) = rms(q)^-1 / sqrt(HD): folds in the score scale
                        sq4 = vec.tile([1, 4 * T], f32, tag="s4", name=f"sq4_{jb}")
                        nc.scalar.activation(sq4[:], ssq, AF.Sqrt, bias=epsB[:], scale=1.0)
                        invq = vec.tile([1, 4 * T], f32r, tag="inv4", name=f"invq{jb}")
                        nc.vector.reciprocal(invq[:], sq4[:])
                        invbq = gn[:, 4 * T:8 * T]
                        nc.tensor.matmul(invbq[:, 0:512], ones1_r[:], invq[:, 0:512], start=True, stop=True,
                                         skip_group_check=True)
                        nc.tensor.matmul(invbq[:, 512:1024], ones1_r[:], invq[:, 512:1024], start=True, stop=True,
                                         skip_group_check=True)
                        qtw = rp.tile([128, 4 * T], f32, tag="r4", name=f"qtw{jb}", bufs=4)
                        nc.vector.tensor_scalar(qtw[:], q4[:], qnw[:], None, ALU.mult)
                        cs1r = bass.AP(cs1.tensor, 0, [[T, 128], [0, 4], [1, T]])
                        cs2r = bass.AP(cs2.tensor, 0, [[T, 128], [0, 4], [1, T]])
                        qA = rp.tile([128, 4 * T], f32, tag="r4", name=f"qA{jb}", bufs=4)
                        nc.vector.tensor_tensor(qA[:], qtw[:], cs1r, ALU.mult)
                        qB = rp.tile([128, 4 * T], f32, tag="r4", name=f"qB{jb}", bufs=4)
                        nc.vector.tensor_tensor(qB[:], qtw[:], cs2r, ALU.mult)
                        qR = rp.tile([128, 4 * T], f32, tag="r4", name=f"qR{jb}", bufs=4)
                        qAh = rp.tile([64, 4 * T], f32, tag="rh", name=f"qAh{jb}", bufs=2)
                        nc.vector.tensor_copy(qAh[:], qA[64:128, :])
                        nc.vector.tensor_tensor(qR[0:64, :], qA[0:64, :], qAh[:], ALU.subtract)
                        qBh = rp.tile([64, 4 * T], f32, tag="rh", name=f"qBh{jb}", bufs=2)
                        nc.vector.tensor_copy(qBh[:], qB[64:128, :])
                        nc.vector.tensor_tensor(qR[64:128, :], qBh[:], qB[0:64, :], ALU.add)
                        nc.vector.tensor_tensor(qT[:, T * jb:T * (jb + 4)], qR[:], invbq, ALU.mult)

            if DEBUG:
                for j in range(NH):
                    nc.sync.dma_start(dbg_q[128 * j:128 * (j + 1), :], qT[:, T * j:T * (j + 1)].bitcast(f32))

            # ================= attention =================
            with tc.tile_pool(name="att", bufs=1) as att, \
                    tc.tile_pool(name="kst", bufs=6) as kstp, \
                    tc.tile_pool(name="vst", bufs=6) as vstp, \
                    tc.tile_pool(name="exp", bufs=3) as expp:
                mask_sb = att.tile([128, 16 * T], bf16)
                for t in range(16):
                    nc.sync.dma_start(mask_sb[:, T * t:T * (t + 1)], mask_d[128 * t:128 * (t + 1), :])

                for g in range(NKV):
                    gOut = psum.tile([128, 2048], f32, tag="g", name=f"gOut{g}")
                    outP = gOut[:, 0:4 * T]
                    denP = gOut[0:1, 4 * T:8 * T]  # [1, 1024] in banks 2-3 (disjoint from outP)
                    kch = []
                    for r in range(NC):
                        kc = kstp.tile([128, T], f32r, tag="k", name=f"kch{g}_{r}")
                        nc.sync.dma_start(kc[:], cc_out_k[512 * r + 128 * g:512 * r + 128 * (g + 1), :])
                        kch.append(kc)
                    gSc = psum.tile([128, 2048], f32, tag="g", name=f"gSc{g}")
                    for kt in range(16):
                        r, half = kt // 2, kt % 2
                        ksl = kch[r][:, 128 * half:128 * (half + 1)]
                        vt = vstp.tile([128, 128], bf16, tag="v", name=f"vt{g}_{kt}")
                        nc.sync.dma_start(vt[:], cc_out_v[128 * kt:128 * (kt + 1), 128 * g:128 * (g + 1)])
                        sc = gSc[:, 1024 * (kt % 2):1024 * (kt % 2) + 1024]
                        for h in range(4):
                            nc.tensor.matmul(sc[:, T * h:T * (h + 1)], ksl,
                                             qT[:, T * (4 * g + h):T * (4 * g + h + 1)],
                                             start=True, stop=True, skip_group_check=True)
                        ex = expp.tile([128, 4 * T], bf16, tag="e", name=f"ex{g}_{kt}")
                        nc.scalar.activation(ex[:], sc, AF.Exp, bias=0.0, scale=1.0)
                        mrep = bass.AP(mask_sb.tensor, T * kt, [[16 * T, 128], [0, 4], [1, T]])
                        nc.vector.tensor_tensor(ex[:], ex[:], mrep, ALU.mult)
                        for h in range(4):
                            mm = nc.tensor.matmul(outP[:, T * h:T * (h + 1)], vt[:], ex[:, T * h:T * (h + 1)],
                                                  start=(kt == 0 and h % 2 == 0), stop=(kt == 15),
                                                  skip_group_check=True)
                            if kt == 0:
                                if h % 2 == 0:
                                    bank_start_o = mm
                                else:
                                    add_dep_helper(mm.ins, bank_start_o.ins, sync=False,
                                                   reason="psum bank pair order")
                        for h in range(4):
                            mm = nc.tensor.matmul(denP[0:1, T * h:T * (h + 1)], ones_b[:], ex[:, T * h:T * (h + 1)],
                                                  start=(kt == 0 and h % 2 == 0), stop=(kt == 15),
                                                  skip_group_check=True)
                            if kt == 0:
                                if h % 2 == 0:
                                    bank_start_d = mm
                                else:
                                    add_dep_helper(mm.ins, bank_start_d.ins, sync=False,
                                                   reason="psum bank pair order")
                    rec = vec.tile([1, 4 * T], f32r, tag="rec", name=f"rec{g}")
                    nc.vector.reciprocal(rec[:], denP)
                    bc = gSc[:, 0:4 * T]
                    nc.tensor.matmul(bc[:, 0:512], ones1_r[:], rec[:, 0:512], start=True, stop=True,
                                     skip_group_check=True)
                    nc.tensor.matmul(bc[:, 512:1024], ones1_r[:], rec[:, 512:1024], start=True, stop=True,
                                     skip_group_check=True)
                    # DVE can read only one PSUM operand: stage the broadcast in SBUF
                    bcS = vec.tile([128, 4 * T], f32, tag="bcS", name=f"bcS{g}")
                    nc.vector.tensor_copy(bcS[:], bc)
                    nc.vector.tensor_tensor(attnT[:, 4 * T * g:4 * T * (g + 1)], outP, bcS[:], ALU.mult)

            # ================= o proj + residual =================
            gO1 = psum.tile([128, 2048], f32, tag="g", name="gO1")
            gO2 = psum.tile([128, 2048], f32, tag="g", name="gO2")
            for i in range(DT):
                wot = wpool.tile([128, 2048], f32r, tag="w", name=f"wo{i}")
                nc.sync.dma_start(wot[:], wo_d[128 * i:128 * (i + 1), :])
                for j in range(DT):
                    gdst = gO1 if j < 8 else gO2
                    mm = nc.tensor.matmul(gdst[:, T * (j % 8):T * (j % 8 + 1)],
                                          wot[:, 128 * j:128 * (j + 1)],
                                          attnT[:, T * i:T * (i + 1)],
                                          start=(i == 0 and j % 2 == 0), stop=(i == DT - 1),
                                          skip_group_check=True)
                    if i == 0:
                        if j % 2 == 0:
                            bank_start = mm
                        else:
                            add_dep_helper(mm.ins, bank_start.ins, sync=False,
                                           reason="psum bank pair order")
            for j in range(DT):
                gdst = gO1 if j < 8 else gO2
                nc.vector.tensor_tensor(h1T[:, T * j:T * (j + 1)], gdst[:, T * (j % 8):T * (j % 8 + 1)],
                                        xT[:, T * j:T * (j + 1)], ALU.add)

            if DEBUG:
                for j in range(NH):
                    nc.sync.dma_start(dbg_at[128 * j:128 * (j + 1), :], attnT[:, T * j:T * (j + 1)].bitcast(f32))
                for j in range(DT):
                    nc.sync.dma_start(dbg_h1[128 * j:128 * (j + 1), :], h1T[:, T * j:T * (j + 1)].bitcast(f32))

            # ================= mu guidance =================
            gM1 = psum.tile([128, 2048], f32, tag="g", name="gM1")
            gM2 = psum.tile([128, 2048], f32, tag="g", name="gM2")
            for i in range(DT):
                wmt = wpool.tile([128, 2048], f32r, tag="w", name=f"wmu{i}")
                nc.sync.dma_start(wmt[:], wmuT_d[128 * i:128 * (i + 1), :])
                for j in range(DT):
                    gdst = gM1 if j < 8 else gM2
                    mm = nc.tensor.matmul(gdst[:, T * (j % 8):T * (j % 8 + 1)],
                                          wmt[:, 128 * j:128 * (j + 1)],
                                          h1T[:, T * i:T * (i + 1)],
                                          start=(i == 0 and j % 2 == 0), stop=(i == DT - 1),
                                          skip_group_check=True)
                    if i == 0:
                        if j % 2 == 0:
                            bank_start = mm
                        else:
                            add_dep_helper(mm.ins, bank_start.ins, sync=False,
                                           reason="psum bank pair order")
            for j in range(DT):
                gdst = gM1 if j < 8 else gM2
                mu_sb = sqp.tile([128, T], f32, tag="mu", name=f"mu{j}", bufs=2)
                nc.vector.tensor_scalar(mu_sb[:], gdst[:, T * (j % 8):T * (j % 8 + 1)],
                                        muclip[:, j:j + 1], None, ALU.add)
                nc.sync.dma_start(muout_d[128 * j:128 * (j + 1), :], mu_sb[:])

            # ================= post norm + MLP =================
            with tc.tile_pool(name="mlp", bufs=1) as mlp:
                gP = psum.tile([128, 2048], f32, tag="g", name="gP")
                ssP = gP[0:1, 0:T]
                for i in range(DT):
                    hsq = sqp.tile([128, T], f32r, tag="sq", name=f"hsq{i}")
                    nc.vector.tensor_tensor(hsq[:], h1T[:, T * i:T * (i + 1)].bitcast(f32),
                                            h1T[:, T * i:T * (i + 1)].bitcast(f32), ALU.mult)
                    nc.tensor.matmul(ssP, ones_r[:], hsq[:], start=(i == 0), stop=(i == DT - 1),
                                     skip_group_check=True)
                invP = rnorm_vec(ssP, epsA, 1.0 / D)
                invbP = gP[:, T:2 * T]
                nc.tensor.matmul(invbP, ones1_r[:], invP[:], start=True, stop=True,
                                 skip_group_check=True)
                h2T = mlp.tile([128, DT * T], f32r)
                for i in range(DT):
                    nc.vector.scalar_tensor_tensor(
                        h2T[:, T * i:T * (i + 1)], h1T[:, T * i:T * (i + 1)].bitcast(f32),
                        lnpost[:, i:i + 1], invbP, ALU.mult, ALU.mult)

                # gate/up projection in 2 passes of 8 EI-tiles
                actT = mlp.tile([128, DT * T], f32r)
                for p in range(2):
                    gG = psum.tile([128, 2048], f32, tag="g", name=f"gG{p}")
                    gU = psum.tile([128, 2048], f32, tag="g", name=f"gU{p}")
                    for i in range(DT):
                        wgt = wpool.tile([128, 2048], f32r, tag="w", name=f"wgu{p}_{i}")
                        nc.sync.dma_start(wgt[:, 0:1024],
                                          wgu_d[128 * i:128 * (i + 1), 1024 * p:1024 * (p + 1)])
                        nc.sync.dma_start(wgt[:, 1024:2048],
                                          wgu_d[128 * i:128 * (i + 1), EI + 1024 * p:EI + 1024 * (p + 1)])
                        for jj in range(8):
                            mmg = nc.tensor.matmul(gG[:, T * jj:T * (jj + 1)],
                                                   wgt[:, 128 * jj:128 * (jj + 1)],
                                                   h2T[:, T * i:T * (i + 1)],
                                                   start=(i == 0 and jj % 2 == 0), stop=(i == DT - 1),
                                                   skip_group_check=True)
                            mmu = nc.tensor.matmul(gU[:, T * jj:T * (jj + 1)],
                                                   wgt[:, 1024 + 128 * jj:1024 + 128 * (jj + 1)],
                                                   h2T[:, T * i:T * (i + 1)],
                                                   start=(i == 0 and jj % 2 == 0), stop=(i == DT - 1),
                                                   skip_group_check=True)
                            if i == 0:
                                if jj % 2 == 0:
                                    bank_start_g, bank_start_u = mmg, mmu
                                else:
                                    add_dep_helper(mmg.ins, bank_start_g.ins, sync=False,
                                                   reason="psum bank pair order")
                                    add_dep_helper(mmu.ins, bank_start_u.ins, sync=False,
                                                   reason="psum bank pair order")
                    for jj in range(8):
                        sg = sqp.tile([128, T], f32, tag="sg", name=f"sg{p}_{jj}", bufs=2)
                        nc.scalar.activation(sg[:], gG[:, T * jj:T * (jj + 1)], AF.Sigmoid, bias=0.0, scale=1.0)
                        sx = sqp.tile([128, T], f32, tag="sx", name=f"sx{p}_{jj}", bufs=2)
                        nc.vector.tensor_tensor(sx[:], sg[:], gG[:, T * jj:T * (jj + 1)], ALU.mult)
                        nc.vector.tensor_tensor(actT[:, T * (8 * p + jj):T * (8 * p + jj + 1)],
                                                sx[:], gU[:, T * jj:T * (jj + 1)], ALU.mult)

                # down projection + final residual
                gD1 = psum.tile([128, 2048], f32, tag="g", name="gD1")
                gD2 = psum.tile([128, 2048], f32, tag="g", name="gD2")
                for i in range(DT):
                    wdt = wpool.tile([128, 2048], f32r, tag="w", name=f"wdw{i}")
                    nc.sync.dma_start(wdt[:], wdw_d[128 * i:128 * (i + 1), :])
                    for j in range(DT):
                        gdst = gD1 if j < 8 else gD2
                        mm = nc.tensor.matmul(gdst[:, T * (j % 8):T * (j % 8 + 1)],
                                              wdt[:, 128 * j:128 * (j + 1)],
                                              actT[:, T * i:T * (i + 1)],
                                              start=(i == 0 and j % 2 == 0), stop=(i == DT - 1),
                                              skip_group_check=True)
                        if i == 0:
                            if j % 2 == 0:
                                bank_start = mm
                            else:
                                add_dep_helper(mm.ins, bank_start.ins, sync=False,
                                               reason="psum bank pair order")
                for j in range(DT):
                    gdst = gD1 if j < 8 else gD2
                    ho = sqp.tile([128, T], f32, tag="mu", name=f"ho{j}", bufs=2)
                    nc.vector.tensor_tensor(ho[:], gdst[:, T * (j % 8):T * (j % 8 + 1)],
                                            h1T[:, T * j:T * (j + 1)].bitcast(f32), ALU.add)
                    nc.sync.dma_start(hout_d[128 * j:128 * (j + 1), :], ho[:])

    nc.compile()
    return nc


def make_in_maps(inputs):
    hs = np.asarray(inputs["hidden_states"], np.float32)
    positions = np.asarray(inputs["positions"]).astype(np.int64)
    sort_idx = np.asarray(inputs["sort_idx"]).astype(np.int64)
    ln_in = np.asarray(inputs["input_ln_w"], np.float32)
    q_proj_w = np.asarray(inputs["q_proj_w"], np.float32)
    o_proj_w = np.asarray(inputs["o_proj_w"], np.float32)
    k_w = np.asarray(inputs["k_w"], np.float32)
    v_w = np.asarray(inputs["v_w"], np.float32)
    q_norm_w = np.asarray(inputs["q_norm_w"], np.float32)
    k_norm_w = np.asarray(inputs["k_norm_w"], np.float32)
    mu = np.asarray(inputs["mu"], np.float32)
    mu_proj_w = np.asarray(inputs["mu_proj_w"], np.float32)
    post_ln = np.asarray(inputs["post_ln_w"], np.float32)
    gate_up = np.asarray(inputs["gate_up_proj"], np.float32)
    down = np.asarray(inputs["down_proj"], np.float32)

    pos_sorted = positions[sort_idx]
    inv_freq = (1.0 / (np.float32(THETA) ** (np.arange(0, 64, dtype=np.float32) / np.float32(64)))).astype(np.float32)
    wkT = np.ascontiguousarray(k_w.T)
    wvT = np.ascontiguousarray(v_w.T)
    wmuT = np.ascontiguousarray(mu_proj_w.T)
    lnin_r = np.ascontiguousarray(ln_in.reshape(DT, 128).T)
    lnpost_r = np.ascontiguousarray(post_ln.reshape(DT, 128).T)
    mu_r = np.ascontiguousarray(mu.reshape(DT, 128).T)
    qnw_r = np.ascontiguousarray(q_norm_w.reshape(128, 1))
    knw_r = np.ascontiguousarray(k_norm_w.reshape(128, 1))

    in_maps = []
    for c in range(NC):
        S = sort_idx[T * c:T * (c + 1)]
        e = c // 2
        pos_c = positions[S].astype(np.float32)
        ang = pos_c[None, :] * inv_freq[:, None]
        mask = (pos_sorted[:, None] <= positions[S][None, :])
        in_maps.append({
            "xT": np.ascontiguousarray(hs[S].T),
            "cosT": np.cos(ang).astype(np.float32),
            "sinT": np.sin(ang).astype(np.float32),
            "maskb": mask.astype(ml_dtypes.bfloat16),
            "lnw_in": lnin_r,
            "lnw_post": lnpost_r,
            "qnw": qnw_r,
            "knw": knw_r,
            "muvec": mu_r,
            "wq": np.ascontiguousarray(q_proj_w[e]),
            "wo": np.ascontiguousarray(o_proj_w[e]),
            "wgu": np.ascontiguousarray(gate_up[e]),
            "wdw": np.ascontiguousarray(down[e]),
            "wmuT": wmuT,
            "wkT": wkT,
            "wvT": wvT,
        })
    return in_maps, sort_idx


def assemble(results, sort_idx):
    h_full = np.empty((N, D), np.float32)
    mu_full = np.empty((N, D), np.float32)
    for c in range(NC):
        S = sort_idx[T * c:T * (c + 1)]
        h_full[S] = results[c]["h_out"].T
        mu_full[S] = results[c]["mu_out"].T
    return h_full, mu_full


def kernel(**inputs):
    if "nc" not in _cached:
        _cached["nc"] = _build()
    nc = _cached["nc"]
    in_maps, sort_idx = make_in_maps(inputs)
    res = run_bass_kernel_spmd(nc, in_maps, core_ids=list(range(NC)))
    return assemble(res.results, sort_idx)


# revision 21
# speedup vs baseline: 1.0450x; 1.0095x over previous
"""Expert-parallel + token-parallel Trainium2 kernel for ComplexityDecoderLayerV2.

Distribution (8 cores, E=4 experts):
  - Tokens are pre-sorted by expert on the host (sort_idx); core c owns the
    256 sorted tokens S_c = sort_idx[256c:256c+256] (expert e = c//2) and all
    per-token compute for them: input rmsnorm, routed q projection, its slice
    of K/V, all 16 attention heads for its 256 query tokens, routed o
    projection, residuals, mu projection, post norm, and the routed MLP.
  - K/V are computed per-core for the core's tokens, then AllGather'd
    (replacing the reference's gather/scatter with a collective, per the
    sharding hint). Causality over the shuffled token order is enforced with a
    host-built multiplicative {0,1} mask on the scores.
  - Activations are kept feature-major ([feature, token]) on-chip so every
    matmul chains without transposes; host transposes/un-permutes at the
    boundary (pure data movement).
Compute dtype: float32r matmuls (fp32 storage, ~1e-4 matmul rel err) with a
bf16 softmax/V path; all accumulation in fp32 PSUM.
"""

import sys

if "/opt/trn_rl_repo" not in sys.path:
    sys.path.insert(0, "/opt/trn_rl_repo")

import numpy as np
import ml_dtypes

import concourse.bass as bass
import concourse.mybir as mybir
import concourse.tile as tile
from concourse import bacc
from concourse.bass_utils import run_bass_kernel_spmd
from concourse.tile import add_dep_helper

f32 = mybir.dt.float32
f32r = mybir.dt.float32r
bf16 = mybir.dt.bfloat16
AF = mybir.ActivationFunctionType
ALU = mybir.AluOpType

NC = 8            # cores
N = 2048          # tokens
D = 2048          # hidden
NH = 16           # q heads
NKV = 4           # kv heads
HD = 128          # head dim
E = 4             # experts
EI = 2048         # per-expert intermediate
T = N // NC       # tokens per core = 256
DT = D // 128     # 16 feature tiles
EPS = 1e-6
THETA = 10000.0

_cached = {}


def _build():
    nc = bacc.Bacc("TRN2", target_bir_lowering=False, debug=False, num_devices=NC)

    # ---- per-core external inputs ----
    xT_d = nc.dram_tensor("xT", [D, T], f32, kind="ExternalInput")
    cos_d = nc.dram_tensor("cosT", [64, T], f32, kind="ExternalInput")
    sin_d = nc.dram_tensor("sinT", [64, T], f32, kind="ExternalInput")
    mask_d = nc.dram_tensor("maskb", [N, T], bf16, kind="ExternalInput")
    lnin_d = nc.dram_tensor("lnw_in", [128, DT], f32, kind="ExternalInput")
    lnpost_d = nc.dram_tensor("lnw_post", [128, DT], f32, kind="ExternalInput")
    qnw_d = nc.dram_tensor("qnw", [128, 1], f32, kind="ExternalInput")
    knw_d = nc.dram_tensor("knw", [128, 1], f32, kind="ExternalInput")
    mu_d = nc.dram_tensor("muvec", [128, DT], f32, kind="ExternalInput")
    wq_d = nc.dram_tensor("wq", [D, NH * HD], f32r, kind="ExternalInput")
    wo_d = nc.dram_tensor("wo", [NH * HD, D], f32r, kind="ExternalInput")
    wgu_d = nc.dram_tensor("wgu", [D, 2 * EI], f32r, kind="ExternalInput")
    wdw_d = nc.dram_tensor("wdw", [EI, D], f32r, kind="ExternalInput")
    wmuT_d = nc.dram_tensor("wmuT", [D, D], f32r, kind="ExternalInput")
    wkT_d = nc.dram_tensor("wkT", [D, NKV * HD], f32r, kind="ExternalInput")
    wvT_d = nc.dram_tensor("wvT", [D, NKV * HD], f32r, kind="ExternalInput")
    hout_d = nc.dram_tensor("h_out", [D, T], f32, kind="ExternalOutput")
    muout_d = nc.dram_tensor("mu_out", [D, T], f32, kind="ExternalOutput")
    DEBUG = bool(__import__("os").environ.get("KERNEL_DEBUG"))
    if DEBUG:
        dbg_xn = nc.dram_tensor("dbg_xn", [D, T], f32, kind="ExternalOutput")
        dbg_q = nc.dram_tensor("dbg_q", [D, T], f32, kind="ExternalOutput")
        dbg_at = nc.dram_tensor("dbg_at", [D, T], f32, kind="ExternalOutput")
        dbg_h1 = nc.dram_tensor("dbg_h1", [D, T], f32, kind="ExternalOutput")
        dbg_k = nc.dram_tensor("dbg_k", [NKV * HD, T], f32, kind="ExternalOutput")
        dbg_v = nc.dram_tensor("dbg_v", [T, NKV * HD], f32, kind="ExternalOutput")

    from contextlib import ExitStack

    with tile.TileContext(nc) as tc, \
            nc.allow_low_precision(reason="f32r/bf16 compute by design"), \
            ExitStack() as stk:
        rg = [list(range(NC))]

        perm = stk.enter_context(tc.tile_pool(name="perm", bufs=1))
        wpool = stk.enter_context(tc.tile_pool(name="wpool", bufs=6))
        sqp = stk.enter_context(tc.tile_pool(name="sqp", bufs=2))
        rp = stk.enter_context(tc.tile_pool(name="rp", bufs=8))
        vec = stk.enter_context(tc.tile_pool(name="vec", bufs=2))
        pstk = ExitStack()
        psum = pstk.enter_context(tc.tile_pool(name="psA", bufs=2, space="PSUM"))
        dram = stk.enter_context(tc.tile_pool(name="dram", bufs=1, space="DRAM"))

        if True:
            # ---- constants ----
            epsA = perm.tile([1, 1], f32)
            nc.vector.memset(epsA[:], EPS)
            epsB = perm.tile([1, 1], f32)
            nc.vector.memset(epsB[:], 128.0 * EPS)
            ones_f = perm.tile([128, 1], f32)
            nc.vector.memset(ones_f[:], 1.0)
            ones_r = perm.tile([128, 1], f32r)
            nc.vector.tensor_copy(ones_r[:], ones_f[:])
            ones1_f = perm.tile([1, 128], f32)
            nc.vector.memset(ones1_f[:], 1.0)
            ones1_r = perm.tile([1, 128], f32r)
            nc.vector.tensor_copy(ones1_r[:], ones1_f[:])
            ones_b = perm.tile([128, 1], bf16)
            nc.vector.memset(ones_b[:], 1.0)
            lnin = perm.tile([128, DT], f32)
            nc.sync.dma_start(lnin[:], lnin_d[:])
            lnpost = perm.tile([128, DT], f32)
            nc.sync.dma_start(lnpost[:], lnpost_d[:])
            qnw = perm.tile([128, 1], f32)
            nc.sync.dma_start(qnw[:], qnw_d[:])
            knw = perm.tile([128, 1], f32)
            nc.sync.dma_start(knw[:], knw_d[:])
            muv = perm.tile([128, DT], f32)
            nc.sync.dma_start(muv[:], mu_d[:])
            muclip = perm.tile([128, DT], f32)
            nc.vector.tensor_scalar(muclip[:], muv[:], 0.0, 2.0, ALU.max, ALU.min)
            cosT = perm.tile([64, T], f32)
            nc.sync.dma_start(cosT[:], cos_d[:])
            sinT = perm.tile([64, T], f32)
            nc.sync.dma_start(sinT[:], sin_d[:])
            cs1 = perm.tile([128, T], f32)
            nc.vector.tensor_copy(cs1[0:64, :], cosT[:])
            nc.vector.tensor_copy(cs1[64:128, :], sinT[:])
            cs2 = perm.tile([128, T], f32)
            nc.vector.tensor_copy(cs2[0:64, :], sinT[:])
            nc.vector.tensor_copy(cs2[64:128, :], cosT[:])

            # ---- persistent activations ----
            xT = perm.tile([128, DT * T], f32)
            for i in range(DT):
                nc.sync.dma_start(xT[:, T * i:T * (i + 1)], xT_d[128 * i:128 * (i + 1), :])
            qT = perm.tile([128, NH * T], f32r)
            attnT = perm.tile([128, NH * T], f32r)
            h1T = perm.tile([128, DT * T], f32r)

            # collective bounce buffers
            cc_in_k = dram.tile([NKV * HD, T], f32r)
            cc_in_v = dram.tile([T, NKV * HD], bf16)
            cc_out_k = dram.tile([NC * NKV * HD, T], f32r, addr_space="Shared")
            cc_out_v = dram.tile([N, NKV * HD], bf16, addr_space="Shared")

            def rnorm_vec(ss_ap, bias_t, scale):
                """1/sqrt(scale*ss + bias) as a [1, T] f32r vector + [128, T] psum bcast."""
                s_t = vec.tile([1, T], f32, tag="s", name=f"s_{nc.next_id()}")
                nc.scalar.activation(s_t[:], ss_ap, AF.Sqrt, bias=bias_t[:], scale=scale)
                inv_t = vec.tile([1, T], f32r, tag="inv", name=f"inv_{nc.next_id()}")
                nc.vector.reciprocal(inv_t[:], s_t[:])
                return inv_t

            # ================= phase A: input rmsnorm =================
            gA = psum.tile([128, 2048], f32, tag="g", name="gA")
            ssA = gA[0:1, 0:T]
            for i in range(DT):
                xsq = sqp.tile([128, T], f32r, tag="sq", name=f"xsq{i}")
                nc.vector.tensor_tensor(xsq[:], xT[:, T * i:T * (i + 1)], xT[:, T * i:T * (i + 1)], ALU.mult)
                nc.tensor.matmul(ssA, ones_r[:], xsq[:], start=(i == 0), stop=(i == DT - 1),
                                 skip_group_check=True)
            invA = rnorm_vec(ssA, epsA, 1.0 / D)
            invbA = gA[:, T:2 * T]
            nc.tensor.matmul(invbA, ones1_r[:], invA[:], start=True, stop=True,
                             skip_group_check=True)
            with tc.tile_pool(name="pre", bufs=1) as pre:
                xnT = pre.tile([128, DT * T], f32r)
                for i in range(DT):
                    nc.vector.scalar_tensor_tensor(
                        xnT[:, T * i:T * (i + 1)], xT[:, T * i:T * (i + 1)],
                        lnin[:, i:i + 1], invbA, ALU.mult, ALU.mult)

                if DEBUG:
                    for i in range(DT):
                        nc.sync.dma_start(dbg_xn[128 * i:128 * (i + 1), :], xnT[:, T * i:T * (i + 1)].bitcast(f32))

                # ================= phase A: K/V projections (first, to launch AG early) ====
                kraw = pre.tile([128, NKV * T], f32)
                gK = psum.tile([128, 2048], f32, tag="g", name="gK")
                for i in range(DT):
                    wk = wpool.tile([128, 2048], f32r, tag="w", name=f"wk{i}")
                    nc.sync.dma_start(wk[:, 0:NKV * HD], wkT_d[128 * i:128 * (i + 1), :])
                    for j in range(NKV):
                        mm = nc.tensor.matmul(gK[:, T * j:T * (j + 1)], wk[:, 128 * j:128 * (j + 1)],
                                              xnT[:, T * i:T * (i + 1)],
                                              start=(i == 0 and j % 2 == 0), stop=(i == DT - 1),
                                              skip_group_check=True)
                        if i == 0:
                            if j % 2 == 0:
                                bank_start = mm
                            else:
                                add_dep_helper(mm.ins, bank_start.ins, sync=False,
                                               reason="psum bank pair order")
                for j in range(NKV):
                    nc.vector.tensor_copy(kraw[:, T * j:T * (j + 1)], gK[:, T * j:T * (j + 1)])

                # k rmsnorm + rope (all 4 heads batched), then to bounce
                kT_c = pre.tile([128, NKV * T], f32r)
                ksq = sqp.tile([128, NKV * T], f32r, tag="sq4", name="ksq", bufs=2)
                nc.vector.tensor_tensor(ksq[:], kraw[:], kraw[:], ALU.mult)
                gn = psum.tile([128, 2048], f32, tag="g", name="gkn")
                ssk = gn[0:1, 0:NKV * T]
                nc.tensor.matmul(gn[0:1, 0:512], ones_r[:], ksq[:, 0:512], start=True, stop=True,
                                 skip_group_check=True)
                nc.tensor.matmul(gn[0:1, 512:1024], ones_r[:], ksq[:, 512:1024], start=True, stop=True,
                                 skip_group_check=True)
                sk = vec.tile([1, NKV * T], f32, tag="s4", name="sk")
                nc.scalar.activation(sk[:], ssk, AF.Sqrt, bias=epsA[:], scale=1.0 / HD)
                invk = vec.tile([1, NKV * T], f32r, tag="inv4", name="invk")
                nc.vector.reciprocal(invk[:], sk[:])
                invbk = gn[:, NKV * T:2 * NKV * T]
                nc.tensor.matmul(invbk[:, 0:512], ones1_r[:], invk[:, 0:512], start=True, stop=True,
                                 skip_group_check=True)
                nc.tensor.matmul(invbk[:, 512:1024], ones1_r[:], invk[:, 512:1024], start=True, stop=True,
                                 skip_group_check=True)
                ktw = rp.tile([128, NKV * T], f32, tag="r4", name="ktw", bufs=4)
                nc.vector.tensor_scalar(ktw[:], kraw[:], knw[:], None, ALU.mult)
                cs1r = bass.AP(cs1.tensor, 0, [[T, 128], [0, NKV], [1, T]])
                cs2r = bass.AP(cs2.tensor, 0, [[T, 128], [0, NKV], [1, T]])
                kA = rp.tile([128, NKV * T], f32, tag="r4", name="kA", bufs=4)
                nc.vector.tensor_tensor(kA[:], ktw[:], cs1r, ALU.mult)
                kB = rp.tile([128, NKV * T], f32, tag="r4", name="kB", bufs=4)
                nc.vector.tensor_tensor(kB[:], ktw[:], cs2r, ALU.mult)
                kR = rp.tile([128, NKV * T], f32, tag="r4", name="kR", bufs=4)
                kAh = rp.tile([64, NKV * T], f32, tag="rh", name="kAh", bufs=2)
                nc.vector.tensor_copy(kAh[:], kA[64:128, :])
                nc.vector.tensor_tensor(kR[0:64, :], kA[0:64, :], kAh[:], ALU.subtract)
                kBh = rp.tile([64, NKV * T], f32, tag="rh", name="kBh", bufs=2)
                nc.vector.tensor_copy(kBh[:], kB[64:128, :])
                nc.vector.tensor_tensor(kR[64:128, :], kBh[:], kB[0:64, :], ALU.add)
                nc.vector.tensor_tensor(kT_c[:], kR[:], invbk, ALU.mult)
                for j in range(NKV):
                    nc.sync.dma_start(cc_in_k[128 * j:128 * (j + 1), :], kT_c[:, T * j:T * (j + 1)])

                # V projection: token-major [T, 512]
                v_sb = pre.tile([128, 2 * NKV * HD], bf16)
                gV = psum.tile([128, 2048], f32, tag="g", name="gV")
                for i in range(DT):
                    wv = wpool.tile([128, 2048], f32r, tag="w", name=f"wv{i}")
                    nc.sync.dma_start(wv[:, 0:NKV * HD], wvT_d[128 * i:128 * (i + 1), :])
                    for h2 in range(2):
                        nc.tensor.matmul(gV[:, 512 * h2:512 * (h2 + 1)],
                                         xnT[:, T * i + 128 * h2:T * i + 128 * h2 + 128],
                                         wv[:, 0:NKV * HD],
                                         start=(i == 0), stop=(i == DT - 1), skip_group_check=True)
                for h2 in range(2):
                    nc.vector.tensor_copy(v_sb[:, 512 * h2:512 * (h2 + 1)], gV[:, 512 * h2:512 * (h2 + 1)])
                    nc.sync.dma_start(cc_in_v[128 * h2:128 * (h2 + 1), :], v_sb[:, 512 * h2:512 * (h2 + 1)])

                # launch the all-gathers
                nc.gpsimd.collective_compute(
                    "AllGather", ALU.bypass, replica_groups=rg,
                    ins=[cc_in_k[:].opt()], outs=[cc_out_k[:].opt()])
                nc.gpsimd.collective_compute(
                    "AllGather", ALU.bypass, replica_groups=rg,
                    ins=[cc_in_v[:].opt()], outs=[cc_out_v[:].opt()])
                if DEBUG:
                    for j in range(NKV):
                        nc.sync.dma_start(dbg_k[128 * j:128 * (j + 1), :], kT_c[:, T * j:T * (j + 1)].bitcast(f32))
                    vdbg = pre.tile([128, 2 * NKV * HD], f32)
                    for h2 in range(2):
                        nc.vector.tensor_copy(vdbg[:, 512 * h2:512 * (h2 + 1)], v_sb[:, 512 * h2:512 * (h2 + 1)])
                        nc.sync.dma_start(dbg_v[128 * h2:128 * (h2 + 1), :], vdbg[:, 512 * h2:512 * (h2 + 1)])

                # ================= phase A: routed Q projection (overlaps AG) ====
                for half in range(2):
                    gQ = psum.tile([128, 2048], f32, tag="g", name=f"gQ{half}")
                    for i in range(DT):
                        wqt = wpool.tile([128, 2048], f32r, tag="w", name=f"wq{half}_{i}")
                        nc.sync.dma_start(wqt[:, 0:1024], wq_d[128 * i:128 * (i + 1), 1024 * half:1024 * (half + 1)])
                        for jj in range(8):
                            mm = nc.tensor.matmul(gQ[:, T * jj:T * (jj + 1)],
                                                  wqt[:, 128 * jj:128 * (jj + 1)],
                                                  xnT[:, T * i:T * (i + 1)],
                                                  start=(i == 0 and jj % 2 == 0), stop=(i == DT - 1),
                                                  skip_group_check=True)
                            if i == 0:
                                if jj % 2 == 0:
                                    bank_start = mm
                                else:
                                    add_dep_helper(mm.ins, bank_start.ins, sync=False,
                                                   reason="psum bank pair order")
                    # drain + q rmsnorm + rope, 4 heads at a time
                    for g4 in range(2):
                        jb = 8 * half + 4 * g4
                        q4 = sqp.tile([128, 4 * T], f32, tag="qraw", name=f"qraw{jb}", bufs=2)
                        nc.vector.tensor_copy(q4[:], gQ[:, 4 * T * g4:4 * T * (g4 + 1)])
                        qsq = sqp.tile([128, 4 * T], f32r, tag="sq4", name=f"qsq{jb}", bufs=2)
                        nc.vector.tensor_tensor(qsq[:], q4[:], q4[:], ALU.mult)
                        gn = psum.tile([128, 2048], f32, tag="g", name=f"gqn{jb}")
                        ssq = gn[0:1, 0:4 * T]
                        nc.tensor.matmul(gn[0:1, 0:512], ones_r[:], qsq[:, 0:512], start=True, stop=True,
                                         skip_group_check=True)
                        nc.tensor.matmul(gn[0:1, 512:1024], ones_r[:], qsq[:, 512:1024], start=True, stop=True,
                                         skip_group_check=True)
                        # 1/sqrt(ss + 128*eps) = rms(q)^-1 / sqrt(HD): folds in the score scale
                        sq4 = vec.tile([1, 4 * T], f32, tag="s4", name=f"sq4_{jb}")
                        nc.scalar.activation(sq4[:], ssq, AF.Sqrt, bias=epsB[:], scale=1.0)
                        invq = vec.tile([1, 4 * T], f32r, tag="inv4", name=f"invq{jb}")
                        nc.vector.reciprocal(invq[:], sq4[:])
                        invbq = gn[:, 4 * T:8 * T]
                        nc.tensor.matmul(invbq[:, 0:512], ones1_r[:], invq[:, 0:512], start=True, stop=True,
                                         skip_group_check=True)
                        nc.tensor.matmul(invbq[:, 512:1024], ones1_r[:], invq[:, 512:1024], start=True, stop=True,
                                         skip_group_check=True)
                        qtw = rp.tile([128, 4 * T], f32, tag="r4", name=f"qtw{jb}", bufs=4)
                        nc.vector.tensor_scalar(qtw[:], q4[:], qnw[:], None, ALU.mult)
                        cs1r = bass.AP(cs1.tensor, 0, [[T, 128], [0, 4], [1, T]])
                        cs2r = bass.AP(cs2.tensor, 0, [[T, 128], [0, 4], [1, T]])
                        qA = rp.tile([128, 4 * T], f32, tag="r4", name=f"qA{jb}", bufs=4)
                        nc.vector.tensor_tensor(qA[:], qtw[:], cs1r, ALU.mult)
                        qB = rp.tile([128, 4 * T], f32, tag="r4", name=f"qB{jb}", bufs=4)
                        nc.vector.tensor_tensor(qB[:], qtw[:], cs2r, ALU.mult)
                        qR = rp.tile([128, 4 * T], f32, tag="r4", name=f"qR{jb}", bufs=4)
                        qAh = rp.tile([64, 4 * T], f32, tag="rh", name=f"qAh{jb}", bufs=2)
                        nc.vector.tensor_copy(qAh[:], qA[64:128, :])
                        nc.vector.tensor_tensor(qR[0:64, :], qA[0:64, :], qAh[:], ALU.subtract)
                        qBh = rp.tile([64, 4 * T], f32, tag="rh", name=f"qBh{jb}", bufs=2)
                        nc.vector.tensor_copy(qBh[:], qB[64:128, :])
                        nc.vector.tensor_tensor(qR[64:128, :], qBh[:], qB[0:64, :], ALU.add)
                        nc.vector.tensor_tensor(qT[:, T * jb:T * (jb + 4)], qR[:], invbq, ALU.mult)

            if DEBUG:
                for j in range(NH):
                    nc.sync.dma_start(dbg_q[128 * j:128 * (j + 1), :], qT[:, T * j:T * (j + 1)].bitcast(f32))

            # ================= attention =================
            pstk.close()  # release phase-A PSUM banks
            with tc.tile_pool(name="psT", bufs=1, space="PSUM") as psT, \
                    tc.tile_pool(name="att", bufs=1) as att, \
                    tc.tile_pool(name="kst", bufs=6) as kstp, \
                    tc.tile_pool(name="vst", bufs=6) as vstp, \
                    tc.tile_pool(name="exp", bufs=3) as expp:
                mask_sb = att.tile([128, 16 * T], bf16)
                for t in range(16):
                    nc.sync.dma_start(mask_sb[:, T * t:T * (t + 1)], mask_d[128 * t:128 * (t + 1), :])

                for g in range(NKV):
                    gOut = psT.tile([128, 2048], f32, tag="od", name=f"gOut{g}", bufs=1)
                    outP = gOut[:, 0:4 * T]
                    denP = gOut[0:1, 4 * T:8 * T]  # [1, 1024] in banks 2-3 (disjoint from outP)
                    kch = []
                    for r in range(NC):
                        kc = kstp.tile([128, T], f32r, tag="k", name=f"kch{g}_{r}")
                        nc.sync.dma_start(kc[:], cc_out_k[512 * r + 128 * g:512 * r + 128 * (g + 1), :])
                        kch.append(kc)
                    for kt in range(16):
                        r, half = kt // 2, kt % 2
                        ksl = kch[r][:, 128 * half:128 * (half + 1)]
                        vt = vstp.tile([128, 128], bf16, tag="v", name=f"vt{g}_{kt}")
                        nc.sync.dma_start(vt[:], cc_out_v[128 * kt:128 * (kt + 1), 128 * g:128 * (g + 1)])
                        scT = psT.tile([128, 1024], f32, tag="sc", name=f"sc{g}_{kt}", bufs=2)
                        sc = scT[:]
                        for h in range(4):
                            nc.tensor.matmul(sc[:, T * h:T * (h + 1)], ksl,
                                             qT[:, T * (4 * g + h):T * (4 * g + h + 1)],
                                             start=True, stop=True, skip_group_check=True)
                        ex = expp.tile([128, 4 * T], bf16, tag="e", name=f"ex{g}_{kt}")
                        nc.scalar.activation(ex[:], sc, AF.Exp, bias=0.0, scale=1.0)
                        mrep = bass.AP(mask_sb.tensor, T * kt, [[16 * T, 128], [0, 4], [1, T]])
                        nc.vector.tensor_tensor(ex[:], ex[:], mrep, ALU.mult)
                        for h in range(4):
                            mm = nc.tensor.matmul(outP[:, T * h:T * (h + 1)], vt[:], ex[:, T * h:T * (h + 1)],
                                                  start=(kt == 0 and h % 2 == 0), stop=(kt == 15),
                                                  skip_group_check=True)
                            if kt == 0:
                                if h % 2 == 0:
                                    bank_start_o = mm
                                else:
                                    add_dep_helper(mm.ins, bank_start_o.ins, sync=False,
                                                   reason="psum bank pair order")
                        for pair in range(2):
                            nc.tensor.matmul(denP[0:1, 512 * pair:512 * (pair + 1)], ones_b[:],
                                             ex[:, 512 * pair:512 * (pair + 1)],
                                             start=(kt == 0), stop=(kt == 15), skip_group_check=True)
                    rec = vec.tile([1, 4 * T], f32r, tag="rec", name=f"rec{g}")
                    nc.vector.reciprocal(rec[:], denP)
                    bcP = psT.tile([128, 1024], f32, tag="sc", name=f"bcP{g}", bufs=2)
                    bc = bcP[:]
                    nc.tensor.matmul(bc[:, 0:512], ones1_r[:], rec[:, 0:512], start=True, stop=True,
                                     skip_group_check=True)
                    nc.tensor.matmul(bc[:, 512:1024], ones1_r[:], rec[:, 512:1024], start=True, stop=True,
                                     skip_group_check=True)
                    # DVE can read only one PSUM operand: stage the broadcast in SBUF
                    bcS = vec.tile([128, 4 * T], f32, tag="bcS", name=f"bcS{g}")
                    nc.vector.tensor_copy(bcS[:], bc)
                    nc.vector.tensor_tensor(attnT[:, 4 * T * g:4 * T * (g + 1)], outP, bcS[:], ALU.mult)

            # ================= o proj + residual =================
            psum = stk.enter_context(tc.tile_pool(name="psC", bufs=2, space="PSUM"))
            gO1 = psum.tile([128, 2048], f32, tag="g", name="gO1")
            gO2 = psum.tile([128, 2048], f32, tag="g", name="gO2")
            for i in range(DT):
                wot = wpool.tile([128, 2048], f32r, tag="w", name=f"wo{i}")
                nc.sync.dma_start(wot[:], wo_d[128 * i:128 * (i + 1), :])
                for j in range(DT):
                    gdst = gO1 if j < 8 else gO2
                    mm = nc.tensor.matmul(gdst[:, T * (j % 8):T * (j % 8 + 1)],
                                          wot[:, 128 * j:128 * (j + 1)],
                                          attnT[:, T * i:T * (i + 1)],
                                          start=(i == 0 and j % 2 == 0), stop=(i == DT - 1),
                                          skip_group_check=True)
                    if i == 0:
                        if j % 2 == 0:
                            bank_start = mm
                        else:
                            add_dep_helper(mm.ins, bank_start.ins, sync=False,
                                           reason="psum bank pair order")
            for j in range(DT):
                gdst = gO1 if j < 8 else gO2
                nc.vector.tensor_tensor(h1T[:, T * j:T * (j + 1)], gdst[:, T * (j % 8):T * (j % 8 + 1)],
                                        xT[:, T * j:T * (j + 1)], ALU.add)

            if DEBUG:
                for j in range(NH):
                    nc.sync.dma_start(dbg_at[128 * j:128 * (j + 1), :], attnT[:, T * j:T * (j + 1)].bitcast(f32))
                for j in range(DT):
                    nc.sync.dma_start(dbg_h1[128 * j:128 * (j + 1), :], h1T[:, T * j:T * (j + 1)].bitcast(f32))

            # ================= mu guidance =================
            gM1 = psum.tile([128, 2048], f32, tag="g", name="gM1")
            gM2 = psum.tile([128, 2048], f32, tag="g", name="gM2")
            for i in range(DT):
                wmt = wpool.tile([128, 2048], f32r, tag="w", name=f"wmu{i}")
                nc.sync.dma_start(wmt[:], wmuT_d[128 * i:128 * (i + 1), :])
                for j in range(DT):
                    gdst = gM1 if j < 8 else gM2
                    mm = nc.tensor.matmul(gdst[:, T * (j % 8):T * (j % 8 + 1)],
                                          wmt[:, 128 * j:128 * (j + 1)],
                                          h1T[:, T * i:T * (i + 1)],
                                          start=(i == 0 and j % 2 == 0), stop=(i == DT - 1),
                                          skip_group_check=True)
                    if i == 0:
                        if j % 2 == 0:
                            bank_start = mm
                        else:
                            add_dep_helper(mm.ins, bank_start.ins, sync=False,
                                           reason="psum bank pair order")
            for j in range(DT):
                gdst = gM1 if j < 8 else gM2
                mu_sb = sqp.tile([128, T], f32, tag="mu", name=f"mu{j}", bufs=2)
                nc.vector.tensor_scalar(mu_sb[:], gdst[:, T * (j % 8):T * (j % 8 + 1)],
                                        muclip[:, j:j + 1], None, ALU.add)
                nc.sync.dma_start(muout_d[128 * j:128 * (j + 1), :], mu_sb[:])

            # ================= post norm + MLP =================
            with tc.tile_pool(name="mlp", bufs=1) as mlp:
                gP = psum.tile([128, 2048], f32, tag="g", name="gP")
                ssP = gP[0:1, 0:T]
                for i in range(DT):
                    hsq = sqp.tile([128, T], f32r, tag="sq", name=f"hsq{i}")
                    nc.vector.tensor_tensor(hsq[:], h1T[:, T * i:T * (i + 1)].bitcast(f32),
                                            h1T[:, T * i:T * (i + 1)].bitcast(f32), ALU.mult)
                    nc.tensor.matmul(ssP, ones_r[:], hsq[:], start=(i == 0), stop=(i == DT - 1),
                                     skip_group_check=True)
                invP = rnorm_vec(ssP, epsA, 1.0 / D)
                invbP = gP[:, T:2 * T]
                nc.tensor.matmul(invbP, ones1_r[:], invP[:], start=True, stop=True,
                                 skip_group_check=True)
                h2T = mlp.tile([128, DT * T], f32r)
                for i in range(DT):
                    nc.vector.scalar_tensor_tensor(
                        h2T[:, T * i:T * (i + 1)], h1T[:, T * i:T * (i + 1)].bitcast(f32),
                        lnpost[:, i:i + 1], invbP, ALU.mult, ALU.mult)

                # gate/up projection in 2 passes of 8 EI-tiles
                actT = mlp.tile([128, DT * T], f32r)
                for p in range(2):
                    gG = psum.tile([128, 2048], f32, tag="g", name=f"gG{p}")
                    gU = psum.tile([128, 2048], f32, tag="g", name=f"gU{p}")
                    for i in range(DT):
                        wgt = wpool.tile([128, 2048], f32r, tag="w", name=f"wgu{p}_{i}")
                        nc.sync.dma_start(wgt[:, 0:1024],
                                          wgu_d[128 * i:128 * (i + 1), 1024 * p:1024 * (p + 1)])
                        nc.sync.dma_start(wgt[:, 1024:2048],
                                          wgu_d[128 * i:128 * (i + 1), EI + 1024 * p:EI + 1024 * (p + 1)])
                        for jj in range(8):
                            mmg = nc.tensor.matmul(gG[:, T * jj:T * (jj + 1)],
                                                   wgt[:, 128 * jj:128 * (jj + 1)],
                                                   h2T[:, T * i:T * (i + 1)],
                                                   start=(i == 0 and jj % 2 == 0), stop=(i == DT - 1),
                                                   skip_group_check=True)
                            mmu = nc.tensor.matmul(gU[:, T * jj:T * (jj + 1)],
                                                   wgt[:, 1024 + 128 * jj:1024 + 128 * (jj + 1)],
                                                   h2T[:, T * i:T * (i + 1)],
                                                   start=(i == 0 and jj % 2 == 0), stop=(i == DT - 1),
                                                   skip_group_check=True)
                            if i == 0:
                                if jj % 2 == 0:
                                    bank_start_g, bank_start_u = mmg, mmu
                                else:
                                    add_dep_helper(mmg.ins, bank_start_g.ins, sync=False,
                                                   reason="psum bank pair order")
                                    add_dep_helper(mmu.ins, bank_start_u.ins, sync=False,
                                                   reason="psum bank pair order")
                    for jj in range(8):
                        sg = sqp.tile([128, T], f32, tag="sg", name=f"sg{p}_{jj}", bufs=2)
                        nc.scalar.activation(sg[:], gG[:, T * jj:T * (jj + 1)], AF.Sigmoid, bias=0.0, scale=1.0)
                        sx = sqp.tile([128, T], f32, tag="sx", name=f"sx{p}_{jj}", bufs=2)
                        nc.vector.tensor_tensor(sx[:], sg[:], gG[:, T * jj:T * (jj + 1)], ALU.mult)
                        nc.vector.tensor_tensor(actT[:, T * (8 * p + jj):T * (8 * p + jj + 1)],
                                                sx[:], gU[:, T * jj:T * (jj + 1)], ALU.mult)

                # down projection + final residual
                gD1 = psum.tile([128, 2048], f32, tag="g", name="gD1")
                gD2 = psum.tile([128, 2048], f32, tag="g", name="gD2")
                for i in range(DT):
                    wdt = wpool.tile([128, 2048], f32r, tag="w", name=f"wdw{i}")
                    nc.sync.dma_start(wdt[:], wdw_d[128 * i:128 * (i + 1), :])
                    for j in range(DT):
                        gdst = gD1 if j < 8 else gD2
                        mm = nc.tensor.matmul(gdst[:, T * (j % 8):T * (j % 8 + 1)],
                                              wdt[:, 128 * j:128 * (j + 1)],
                                              actT[:, T * i:T * (i + 1)],
                                              start=(i == 0 and j % 2 == 0), stop=(i == DT - 1),
                                              skip_group_check=True)
                        if i == 0:
                            if j % 2 == 0:
                                bank_start = mm
                            else:
                                add_dep_helper(mm.ins, bank_start.ins, sync=False,
                                               reason="psum bank pair order")
                for j in range(DT):
                    gdst = gD1 if j < 8 else gD2
                    ho = sqp.tile([128, T], f32, tag="mu", name=f"ho{j}", bufs=2)
                    nc.vector.tensor_tensor(ho[:], gdst[:, T * (j % 8):T * (j % 8 + 1)],
                                            h1T[:, T * j:T * (j + 1)].bitcast(f32), ALU.add)
                    nc.sync.dma_start(hout_d[128 * j:128 * (j + 1), :], ho[:])

    nc.compile()
    return nc


def make_in_maps(inputs):
    hs = np.asarray(inputs["hidden_states"], np.float32)
    positions = np.asarray(inputs["positions"]).astype(np.int64)
    sort_idx = np.asarray(inputs["sort_idx"]).astype(np.int64)
    ln_in = np.asarray(inputs["input_ln_w"], np.float32)
    q_proj_w = np.asarray(inputs["q_proj_w"], np.float32)
    o_proj_w = np.asarray(inputs["o_proj_w"], np.float32)
    k_w = np.asarray(inputs["k_w"], np.float32)
    v_w = np.asarray(inputs["v_w"], np.float32)
    q_norm_w = np.asarray(inputs["q_norm_w"], np.float32)
    k_norm_w = np.asarray(inputs["k_norm_w"], np.float32)
    mu = np.asarray(inputs["mu"], np.float32)
    mu_proj_w = np.asarray(inputs["mu_proj_w"], np.float32)
    post_ln = np.asarray(inputs["post_ln_w"], np.float32)
    gate_up = np.asarray(inputs["gate_up_proj"], np.float32)
    down = np.asarray(inputs["down_proj"], np.float32)

    pos_sorted = positions[sort_idx]
    inv_freq = (1.0 / (np.float32(THETA) ** (np.arange(0, 64, dtype=np.float32) / np.float32(64)))).astype(np.float32)
    wkT = np.ascontiguousarray(k_w.T)
    wvT = np.ascontiguousarray(v_w.T)
    wmuT = np.ascontiguousarray(mu_proj_w.T)
    lnin_r = np.ascontiguousarray(ln_in.reshape(DT, 128).T)
    lnpost_r = np.ascontiguousarray(post_ln.reshape(DT, 128).T)
    mu_r = np.ascontiguousarray(mu.reshape(DT, 128).T)
    qnw_r = np.ascontiguousarray(q_norm_w.reshape(128, 1))
    knw_r = np.ascontiguousarray(k_norm_w.reshape(128, 1))

    in_maps = []
    for c in range(NC):
        S = sort_idx[T * c:T * (c + 1)]
        e = c // 2
        pos_c = positions[S].astype(np.float32)
        ang = pos_c[None, :] * inv_freq[:, None]
        mask = (pos_sorted[:, None] <= positions[S][None, :])
        in_maps.append({
            "xT": np.ascontiguousarray(hs[S].T),
            "cosT": np.cos(ang).astype(np.float32),
            "sinT": np.sin(ang).astype(np.float32),
            "maskb": mask.astype(ml_dtypes.bfloat16),
            "lnw_in": lnin_r,
            "lnw_post": lnpost_r,
            "qnw": qnw_r,
            "knw": knw_r,
            "muvec": mu_r,
            "wq": np.ascontiguousarray(q_proj_w[e]),
            "wo": np.ascontiguousarray(o_proj_w[e]),
            "wgu": np.ascontiguousarray(gate_up[e]),
            "wdw": np.ascontiguousarray(down[e]),
            "wmuT": wmuT,
            "wkT": wkT,
            "wvT": wvT,
        })
    return in_maps, sort_idx


def assemble(results, sort_idx):
    h_full = np.empty((N, D), np.float32)
    mu_full = np.empty((N, D), np.float32)
    for c in range(NC):
        S = sort_idx[T * c:T * (c + 1)]
        h_full[S] = results[c]["h_out"].T
        mu_full[S] = results[c]["mu_out"].T
    return h_full, mu_full


def kernel(**inputs):
    if "nc" not in _cached:
        _cached["nc"] = _build()
    nc = _cached["nc"]
    in_maps, sort_idx = make_in_maps(inputs)
    res = run_bass_kernel_spmd(nc, in_maps, core_ids=list(range(NC)))
    return assemble(res.results, sort_idx)
